# revision 36
# baseline (speedup 1.0000x reference)
"""Trainium2 Bass kernel for nn_EpiNN_aaindex (pairwise-MLP GNN reduction).

Math (per batch b):
  x1 = emb@tw + tb                               (host)
  X[i,d] = emb[i*64+d] * tw[i*64+d]              (L=256, D=64; host)
  s_ij = MLP(concat[(x_i+x_j)/2, |x_i-x_j|])     (64->16->1, LeakyReLU 0.01)
  out_b = x1 + scale * sum_{i<j} s_ij

Strategy: 8 cores, 4 batches/core (data parallel over B=32).
Exact upper-triangle enumeration via cyclic offsets o=1..128:
pairs (i, (i+o) mod 256) for o=1..127 cover each unordered pair once;
o=128 covers each of its 128 pairs twice (weighted 0.5 on the host).

Host prep per batch: X2T = (emb[:, :-1].reshape(L, D) * tw2d).T  [64, 256]
  XALL [128, 1152] bf16: [X2T|X2T (512) | XSo (320) | XSe (320)]
    XSo: top half rows = X2T cyclic-shift 1, bottom = shift 65
    XSe: shifts 2 / 66
  XF8 [128, 1152] fp8e4: [X2T(256)|XSo(320) | X2T(256)|XSe(320)] - for the
    DoubleRow S-term matmuls.

Device per iteration it (c0 = 4*it), quarters q0..q3 of 256 pairs each
(q0 = offsets 4it+1|+65, q1 = 4it+3|+67, q2 = 4it+2|+66, q3 = 4it+4|+68):
  A2 [128, 1024] bf16 = |XU - XS[c0 window]|: one fused tensor_tensor
    subtract (on the DVE for even iterations, on GPSIMD for odd ones --
    walrus accepts TensorTensor on Pool, not TensorScalarPtr) + one DVE
    bitand (u16 4x mode) for the abs.
  P1 [128, 1024] f32 psum:
    bf16 matmul  P1 += WBSstack @ A2            (128-contract, 2x512 cols)
    fp8 DoubleRow P1 += WAS@X2T + WAS@XS-window (4 mms, 256 cols @ 2x rate)
  H1 = Lrelu(P1 + b1)  [128, 1024] bf16   (ACT, bias=B1S)
  P2: layer-2 outputs of iteration PAIRS share one [128, 1024]-capable psum
    tile; per iteration 4 bf16 matmuls place each quarter's 16-dim output at
    psum rows 32q (top) / 32q+16 (bottom) -> all 128 rows used.
  act2 = Lrelu(P2 + b2) summed over pairs: split between the ACT engine
    (activation + accum_out; 1 pair-group of 7 + both solo groups) and the
    DVE (tensor_scalar + scalar_tensor_tensor max(x, 0.01x) with
    accum_out) so that ACT/DVE/PE/Pool all sit near 80-87% busy.

Layer-2 weights stay bf16: quantizing w2 to fp8 injects noise into pre2,
and E[lrelu(x + noise)] > lrelu(x) (convexity) -> a systematic positive
output bias (~+1.2 measured). fp8 on the layer-1 S-term is bias-free.

Software-pipelined emission with 3-iteration skew: layer-2 of iteration
k-3 is emitted after layer-1 of iteration k so the PE queue never stalls
on act1.

Final combine on host: out = x1 + scale*(w3tile . ACC - 0.5*dup128 + b3*32640).
"""
import numpy as np

L, D = 256, 64
B_PER_CORE = 4
N_CORES = 8
NPAIRS = 32640  # 256*255/2

_CACHE = {}
import os as _os
_os  # env knobs read in _build_program too
N_ITERS = int(_os.environ.get("EPINN_ITERS", "16"))
N_BATCH = int(_os.environ.get("EPINN_BATCH", str(B_PER_CORE)))
N_RUN_CORES = int(_os.environ.get("EPINN_CORES", str(N_CORES)))
USE_FP8 = _os.environ.get("EPINN_FP8", "1") == "1"
USE_FP8_L2 = _os.environ.get("EPINN_FP8_L2", "0") == "1"
N_DVE_ACT2 = int(_os.environ.get("EPINN_DVE_ACT2", "3"))
_ACT_PAIR_SETS = {
    0: (),
    1: (3,),
    2: (1, 5),
    3: (0, 3, 6),
    4: (0, 2, 4, 6),
    5: (0, 1, 3, 5, 6),
    7: (0, 1, 2, 3, 4, 5, 6),
}
ACT_PAIRS = _ACT_PAIR_SETS[int(_os.environ.get("EPINN_ACT_PAIRS", "1"))]
# 0 = A2 subtract always on DVE, 2 = subtract on Pool for odd iterations
# (walrus accepts TensorTensor on Pool but rejects TensorScalarPtr there)
POOL_SUB = int(_os.environ.get("EPINN_POOL_SUB", "2"))
ACT_SOLOS = int(_os.environ.get("EPINN_ACT_SOLOS", "2"))
SKEW = int(_os.environ.get("EPINN_SKEW", "3"))


def _build_program():
    import concourse.bacc as bacc
    import concourse.mybir as mybir
    import concourse.tile as tile
    from concourse.ap import AP
    from contextlib import ExitStack

    f32 = mybir.dt.float32
    bf16 = mybir.dt.bfloat16
    fp8 = mybir.dt.float8e4
    u16 = mybir.dt.uint16
    AF = mybir.ActivationFunctionType
    ALU = mybir.AluOpType
    DR = mybir.MatmulPerfMode.DoubleRow

    nc = bacc.Bacc("TRN2", target_bir_lowering=False, debug=False,
                   num_devices=N_CORES)

    # ---- DRAM parameters (per core) ----
    xall_d = nc.declare_dram_parameter("xall", [N_BATCH, 128, 1152], bf16,
                                       isOutput=False)
    xf8_d = nc.declare_dram_parameter("xf8", [N_BATCH, 128, 1152], fp8,
                                      isOutput=False)
    wk_d = nc.declare_dram_parameter("wk", [128, 1168], bf16, isOutput=False)

    # acc columns: iterations grouped (quads, then a triple, then the last
    # iteration solo -- it holds the double-counted o=128 stream and must stay
    # separable for the host-side correction).
    assert N_ITERS == 16
    GROUPS = [(0, 1), (2, 3), (4, 5), (6, 7), (8, 9), (10, 11), (12, 13),
              (14,), (15,)]
    n_acc = len(GROUPS)
    acc_o = nc.declare_dram_parameter("acc_o", [N_BATCH, 128, n_acc], f32,
                                      isOutput=True)

    with tile.TileContext(nc) as tc, ExitStack() as ctx:
        cpool = ctx.enter_context(tc.tile_pool(name="consts", bufs=1))
        XBUFS = int(_os.environ.get("EPINN_XBUFS", "2"))
        xpool = ctx.enter_context(tc.tile_pool(name="xbufs", bufs=XBUFS))
        apool = ctx.enter_context(tc.tile_pool(name="abufs", bufs=2 + SKEW))
        hpool = ctx.enter_context(tc.tile_pool(name="hbufs", bufs=2 + SKEW))
        JBUFS = int(_os.environ.get("EPINN_JBUFS", "2"))
        jpool = ctx.enter_context(tc.tile_pool(name="junk", bufs=JBUFS))
        opool = ctx.enter_context(tc.tile_pool(name="outs", bufs=2))
        pp1 = ctx.enter_context(tc.tile_pool(name="p1", bufs=3, space="PSUM"))
        pp2 = ctx.enter_context(tc.tile_pool(name="p2", bufs=2, space="PSUM"))

        DUM = cpool.tile([1, 2], f32)
        nc.gpsimd.memset(DUM[:], 0.0)
        nc.scalar.activation(DUM[:], DUM[:], AF.Lrelu, scale=1.0, alpha=0.01)

        # batch-0 inputs first: the first DVE subtract only needs XALL, so
        # its DMA leads; weights follow (needed ~1.5us later by the first mm).
        XT0 = xpool.tile([128, 1152], bf16, tag="xall", name="xall0")
        nc.sync.dma_start(XT0[:], xall_d[0])
        WK = cpool.tile([128, 1168], bf16)
        nc.sync.dma_start(WK[:], wk_d[:])
        XF0 = xpool.tile([128, 1152], fp8, tag="xf8", name="xf80")
        if USE_FP8:
            nc.sync.dma_start(XF0[:], xf8_d[0])
        WBS = WK[:, 0:128]
        W2S = WK[:, 128:640]
        WASB = WK[:, 640:768]
        WAS8 = WK[:, 768:896].bitcast(fp8)
        BP = WK[:, 896:912].bitcast(f32)
        B1S = BP[:, 0:1]
        B2S = BP[:, 1:2]
        B1S64 = BP[:, 2:3]
        B2S64 = BP[:, 3:4]
        B2S64001 = BP[:, 4:5]
        WAS2 = WAS8.rearrange("p (i m) -> p i m", i=2)
        W2D = WK[:, 912:1168].bitcast(fp8)  # [128, 512] = 2 groups x [2, 128]

        for b in range(N_BATCH):
            if b == 0:
                XALL, XF8 = XT0, XF0
            else:
                XALL = xpool.tile([128, 1152], bf16, tag="xall")
                XF8 = xpool.tile([128, 1152], fp8, tag="xf8")
                nc.sync.dma_start(XALL[:], xall_d[b])
                if USE_FP8:
                    nc.sync.dma_start(XF8[:], xf8_d[b])

            ACC = opool.tile([128, n_acc], f32, tag="acc")

            def sub_in0():
                base = XALL[:, 0:512]
                return AP(base.tensor, base.offset,
                          [[1152, 128], [0, 2], [256, 2], [1, 256]])

            def sub_in1(c0):
                base = XALL[:, 512 + c0:512 + c0 + 1]
                return AP(base.tensor, base.offset,
                          [[1152, 128], [320, 2], [2, 2], [1, 256]])

            def dr_rhs(h, c0, s):
                base = XF8[:, 576 * h:576 * h + 1]
                return AP(base.tensor, base.offset,
                          [[1152, 128], [256 + c0 + 2 * s, 2], [1, 256]])

            def bf_rhs(h, c0, s):
                # fallback (no fp8): [X2T-copy | XS-window] via two mms
                base = XALL[:, 0:1]
                xu = AP(base.tensor, base.offset, [[1152, 128], [1, 256]])
                b2 = XALL[:, 512 + 320 * h + c0 + 2 * s:]
                xs = AP(b2.tensor, b2.offset, [[1152, 128], [1, 256]])
                return xu, xs

            git = {it: (gi, j, len(g))
                   for gi, g in enumerate(GROUPS) for j, it in enumerate(g)}
            p2_state = [None]  # current group psum tile

            def emit_l2(it, H1):
                gi, j, glen = git[it]
                if j == 0:
                    p2_state[0] = pp2.tile([128, 512], f32, tag="p2",
                                           name="p2t")
                cur = p2_state[0]
                lo = 256 * j
                if USE_FP8_L2:
                    for g in range(2):
                        w2d = W2D[:, 256 * g:256 * g + 256].rearrange(
                            "p (i m) -> p i m", i=2)
                        nc.tensor.matmul(cur[:, lo:lo + 256], w2d,
                                         H1[:, 512 * g:512 * g + 512].rearrange(
                                             "p (i c) -> p i c", i=2),
                                         start=(g == 0), stop=(g == 1),
                                         perf_mode=DR, skip_group_check=True)
                else:
                    for q in range(4):
                        nc.tensor.matmul(cur[:, lo:lo + 256],
                                         W2S[:, 128 * q:128 * q + 128],
                                         H1[:, 256 * q:256 * q + 256],
                                         start=(q == 0), stop=(q == 3),
                                         skip_group_check=True)
                if j != glen - 1:
                    return
                width = 256 * glen
                HJ = jpool.tile([128, 512], bf16, tag="hj")
                # spread most pair-activations onto the DVE (fused
                # lrelu+accumulate) to unload the ACT engine; the rest use
                # ACT with its built-in accumulator.
                # in fp8 mode P2 holds 64*(pre2 - b2); compute 64*lrelu2
                # (homogeneity) and divide acc by 64 on the host.
                on_dve = ((glen == 2 and gi % 7 not in ACT_PAIRS)
                          or (glen == 1 and (gi - 7) >= ACT_SOLOS))
                if on_dve:
                    T2 = jpool.tile([128, 512], bf16, tag="t2")
                    if USE_FP8_L2:
                        nc.vector.tensor_scalar(
                            out=T2[:, 0:width], in0=cur[:, 0:width],
                            scalar1=0.01, scalar2=B2S64001,
                            op0=ALU.mult, op1=ALU.add)
                        nc.vector.scalar_tensor_tensor(
                            out=HJ[:, 0:width], in0=cur[:, 0:width],
                            scalar=B2S64, in1=T2[:, 0:width],
                            op0=ALU.add, op1=ALU.max,
                            accum_out=ACC[:, gi:gi + 1])
                    else:
                        nc.vector.tensor_scalar(
                            out=T2[:, 0:width], in0=cur[:, 0:width],
                            scalar1=B2S, scalar2=0.01,
                            op0=ALU.add, op1=ALU.mult)
                        nc.vector.scalar_tensor_tensor(
                            out=HJ[:, 0:width], in0=cur[:, 0:width],
                            scalar=B2S, in1=T2[:, 0:width],
                            op0=ALU.add, op1=ALU.max,
                            accum_out=ACC[:, gi:gi + 1])
                else:
                    nc.scalar.activation(HJ[:, 0:width], cur[:, 0:width],
                                         AF.Lrelu,
                                         bias=B2S64 if USE_FP8_L2 else B2S,
                                         scale=1.0, alpha=0.01,
                                         accum_out=ACC[:, gi:gi + 1])

            pending = []  # [(it, H1)] pending layer-2 (2-deep skew)
            for it in range(N_ITERS):
                c0 = 4 * it
                A2 = apool.tile([128, 1024], bf16, tag="a2")
                pool_sub = POOL_SUB == 2 and it % 2 == 1
                eng_sub = nc.gpsimd if pool_sub else nc.vector
                eng_sub.tensor_tensor(
                    out=A2[:].rearrange("p (t s c) -> p t s c", t=2, s=2),
                    in0=sub_in0(), in1=sub_in1(c0), op=ALU.subtract)
                nc.vector.tensor_scalar(
                    out=A2[:].bitcast(u16), in0=A2[:].bitcast(u16),
                    scalar1=0x7FFF, scalar2=None, op0=ALU.bitwise_and)

                # ---- layer 1 ----
                P1 = pp1.tile([128, 1024], f32, tag="p1")
                nc.tensor.matmul(P1[:, 0:512], WBS, A2[:, 0:512],
                                 start=True, stop=False, skip_group_check=True)
                nc.tensor.matmul(P1[:, 512:1024], WBS, A2[:, 512:1024],
                                 start=True, stop=False, skip_group_check=True)
                for h in range(2):
                    for s in range(2):
                        pslice = P1[:, 512 * h + 256 * s:512 * h + 256 * s + 256]
                        if USE_FP8:
                            nc.tensor.matmul(pslice, WAS2, dr_rhs(h, c0, s),
                                             start=False, stop=True,
                                             perf_mode=DR,
                                             skip_group_check=True)
                        else:
                            xu, xs = bf_rhs(h, c0, s)
                            nc.tensor.matmul(pslice, WASB, xu,
                                             start=False, stop=False,
                                             skip_group_check=True)
                            nc.tensor.matmul(pslice, WASB, xs,
                                             start=False, stop=True,
                                             skip_group_check=True)

                # software pipeline: layer 2 of iteration it-2 sits behind
                # this iteration's layer 1 in the PE queue.
                if len(pending) >= SKEW:
                    emit_l2(*pending.pop(0))

                H1 = hpool.tile([128, 1024], fp8 if USE_FP8_L2 else bf16,
                                tag="h1")
                if USE_FP8_L2:
                    nc.scalar.activation(H1[:], P1[:], AF.Lrelu, bias=B1S64,
                                         scale=64.0, alpha=0.01)
                else:
                    nc.scalar.activation(H1[:], P1[:], AF.Lrelu, bias=B1S,
                                         scale=1.0, alpha=0.01)
                pending.append((it, H1))

            for p in pending:
                emit_l2(*p)
            nc.sync.dma_start(acc_o[b], ACC[:])

    nc.compile()
    return nc


def _get_program():
    key = (N_ITERS, N_BATCH, USE_FP8, USE_FP8_L2, N_DVE_ACT2, SKEW,
           ACT_PAIRS, POOL_SUB, ACT_SOLOS,
           _os.environ.get("EPINN_JBUFS", "2"),
           _os.environ.get("EPINN_XBUFS", "2"))
    if key not in _CACHE:
        _CACHE[key] = _build_program()
    return _CACHE[key]


def _get_runner():
    """Build (once) a cached jitted SPMD executable for the program."""
    key = ("runner", N_ITERS, N_BATCH, N_RUN_CORES, USE_FP8, USE_FP8_L2,
           N_DVE_ACT2, ACT_PAIRS, POOL_SUB, ACT_SOLOS)
    if key in _CACHE:
        return _CACHE[key]
    import jax
    import jax.numpy as jnp
    import numpy as _np
    import concourse.mybir as mybir
    from jax.sharding import Mesh, PartitionSpec
    from jax.experimental.shard_map import shard_map
    from concourse import bass2jax
    from concourse.bass2jax import _bass_exec_p, partition_id_tensor

    bass2jax.install_neuronx_cc_hook()
    nc = _get_program()
    n_cores = N_RUN_CORES

    partition_name = (nc.partition_id_tensor.name
                      if nc.partition_id_tensor else None)
    in_names, out_names, out_avals, zero_shapes = [], [], [], []
    for alloc in nc.m.functions[0].allocations:
        if not isinstance(alloc, mybir.MemoryLocationSet):
            continue
        name = alloc.memorylocations[0].name
        if alloc.kind == "ExternalInput":
            if name != partition_name:
                in_names.append(name)
        elif alloc.kind == "ExternalOutput":
            out_names.append(name)
            shape = tuple(alloc.tensor_shape)
            dtype = mybir.dt.np(alloc.dtype)
            out_avals.append(jax.core.ShapedArray(shape, dtype))
            zero_shapes.append((shape, dtype))
    n_params = len(in_names)
    n_outs = len(out_avals)
    all_in_names = list(in_names) + list(out_names)
    if partition_name is not None:
        all_in_names.append(partition_name)
    donate = tuple(range(n_params, n_params + n_outs))

    def _body(*args):
        operands = list(args)
        if partition_name is not None:
            operands.append(partition_id_tensor())
        outs = _bass_exec_p.bind(
            *operands, out_avals=tuple(out_avals), in_names=tuple(all_in_names),
            out_names=tuple(out_names), lowering_input_output_aliases=(),
            sim_require_finite=True, sim_require_nnan=True, nc=nc)
        return tuple(outs)

    devices = jax.devices()[:n_cores]
    mesh = Mesh(_np.asarray(devices), ("core",))
    in_specs = (PartitionSpec("core"),) * (n_params + n_outs)
    out_specs = (PartitionSpec("core"),) * len(out_names)
    sharded = jax.jit(
        shard_map(_body, mesh=mesh, in_specs=in_specs, out_specs=out_specs,
                  check_rep=False),
        donate_argnums=donate, keep_unused=True)

    def run(in_maps):
        concat_in = [
            np.concatenate([np.asarray(in_maps[c][nm]) for c in range(n_cores)],
                           axis=0)
            for nm in in_names
        ]
        concat_zeros = [np.zeros((n_cores * s[0], *s[1:]), d)
                        for (s, d) in zero_shapes]
        out_arrs = sharded(*concat_in, *concat_zeros)
        return [
            {nm: np.asarray(out_arrs[i]).reshape(n_cores, *out_avals[i].shape)[c]
             for i, nm in enumerate(out_names)}
            for c in range(n_cores)
        ]

    _CACHE[key] = run
    return run


def _prep_in_maps(emb, tw, w1, b1, w2, b2):
    import ml_dtypes
    bfl = ml_dtypes.bfloat16
    f8 = ml_dtypes.float8_e4m3

    emb = np.asarray(emb, np.float32)
    tw = np.asarray(tw, np.float32)
    w1 = np.asarray(w1, np.float32)
    b1v = np.asarray(b1, np.float32)
    w2f = np.asarray(w2, np.float32)
    b2v = np.asarray(b2, np.float32)

    w1bt = np.ascontiguousarray(w1[:, 64:].T)          # [64f, 64d]
    w1at = np.ascontiguousarray(0.5 * w1[:, :64].T)    # [64f, 64d]
    wbs = np.zeros((128, 128), np.float32)
    wbs[0:64, 0:64] = w1bt
    wbs[64:128, 64:128] = w1bt
    was = np.zeros((128, 128), np.float32)
    was[0:64, 0:64] = w1at
    was[64:128, 64:128] = w1at
    w2s = np.zeros((128, 512), np.float32)
    for q in range(4):
        w2s[0:64, 128 * q + 32 * q:128 * q + 32 * q + 16] = w2f.T
        w2s[64:128, 128 * q + 32 * q + 16:128 * q + 32 * q + 32] = w2f.T
    wp = np.concatenate([wbs, w2s, was], axis=1)       # [128, 768]
    was8 = np.concatenate([was, was], axis=1)          # [128, 256]
    b1d = np.concatenate([b1v, b1v])
    b2d = np.tile(b2v, 8)
    bp = np.stack([b1d, b2d, 64.0 * b1d, 64.0 * b2d, 0.64 * b2d,
                   np.zeros(128, np.float32), np.zeros(128, np.float32),
                   np.zeros(128, np.float32)], axis=1)  # [128, 8]
    w2d = np.zeros((128, 2, 2, 128), np.float32)       # [p, g, i, m]
    for q in range(4):
        w2d[:, q // 2, q % 2, :] = w2s[:, 128 * q:128 * q + 128]
    w2d = w2d.reshape(128, 512)
    wk = np.zeros((128, 1168 * 2), np.uint8)
    wk[:, 0:1536] = wp.astype(bfl).view(np.uint8)
    wk[:, 1536:1792] = was8.astype(f8).view(np.uint8)
    wk[:, 1792:1824] = bp.astype(np.float32).view(np.uint8)
    wk[:, 1824:2336] = w2d.astype(f8).view(np.uint8)
    wk = wk.view(bfl)                                  # [128, 1168]

    twp = tw[:-1].reshape(L, D)
    idx1 = (np.arange(320) + 1) % 256
    idx65 = (np.arange(320) + 65) % 256
    idx2 = (np.arange(320) + 2) % 256
    idx66 = (np.arange(320) + 66) % 256

    shared = {"wk": wk}
    in_maps = []
    for c in range(N_CORES):
        xall = np.zeros((N_BATCH, 128, 1152), np.float32)
        for b in range(N_BATCH):
            gb = c * B_PER_CORE + b
            x2t = (emb[gb, :L * D].reshape(L, D) * twp).T  # [64, 256]
            x2t2 = np.concatenate([x2t, x2t], axis=0)      # [128, 256]
            xall[b, :, 0:256] = x2t2
            xall[b, :, 256:512] = x2t2
            xall[b, 0:64, 512:832] = x2t[:, idx1]
            xall[b, 64:128, 512:832] = x2t[:, idx65]
            xall[b, 0:64, 832:1152] = x2t[:, idx2]
            xall[b, 64:128, 832:1152] = x2t[:, idx66]
        xf8 = np.zeros((N_BATCH, 128, 1152), np.float32)
        xf8[:, :, 0:256] = xall[:, :, 0:256]
        xf8[:, :, 256:576] = xall[:, :, 512:832]    # XSo
        xf8[:, :, 576:832] = xall[:, :, 0:256]      # X2T again
        xf8[:, :, 832:1152] = xall[:, :, 832:1152]  # XSe
        m = dict(shared)
        m["xall"] = xall.astype(bfl)
        m["xf8"] = xf8.astype(f8)
        in_maps.append(m)
    return in_maps


_FP8_SCALED = USE_FP8_L2


def _finish(core_results, emb, tw, tb, w3, b3, scale):
    emb = np.asarray(emb, np.float64)
    tw = np.asarray(tw, np.float64)
    x1 = emb @ tw + float(tb[0])  # [32]
    w3v = np.asarray(w3, np.float32)[0]       # [16]
    w3t = np.tile(w3v, 8)                     # [128]
    out = np.zeros(32, np.float32)
    for c in range(len(core_results)):
        acc = core_results[c]["acc_o"]        # [4, 128, n_acc]
        for b in range(N_BATCH):
            a = acc[b] / 64.0 if _FP8_SCALED else acc[b]
            tot = float(w3t @ a.sum(axis=1))
            # o=128 stream (rows 112:128 of the last acc col) double counted
            tot -= 0.5 * float(w3v @ a[112:128, -1])
            gb = c * B_PER_CORE + b
            out[gb] = (x1[gb]
                       + float(scale[0]) * (tot + float(b3[0]) * NPAIRS))
    return out


def kernel(emb, tw, tb, w1, b1, w2, b2, w3, b3, scale):
    run = _get_runner()
    in_maps = _prep_in_maps(emb, tw, w1, b1, w2, b2)
    core_results = run(in_maps[:N_RUN_CORES])
    return _finish(core_results, emb, tw, tb, w3, b3, scale)


# revision 38
# speedup vs baseline: 1.0084x; 1.0084x over previous
"""Trainium2 Bass kernel for nn_EpiNN_aaindex (pairwise-MLP GNN reduction).

Math (per batch b):
  x1 = emb@tw + tb                               (host)
  X[i,d] = emb[i*64+d] * tw[i*64+d]              (L=256, D=64; host)
  s_ij = MLP(concat[(x_i+x_j)/2, |x_i-x_j|])     (64->16->1, LeakyReLU 0.01)
  out_b = x1 + scale * sum_{i<j} s_ij

Strategy: 8 cores, 4 batches/core (data parallel over B=32).
Exact upper-triangle enumeration via cyclic offsets o=1..128:
pairs (i, (i+o) mod 256) for o=1..127 cover each unordered pair once;
o=128 covers each of its 128 pairs twice -- the host recomputes that one
offset's 256-pair term exactly in numpy and subtracts half of it.

Host prep per batch: X2T = (emb[:, :-1].reshape(L, D) * tw2d).T  [64, 256]
  XALL [128, 1152] bf16: [X2T|X2T (512) | XSo (320) | XSe (320)]
    XSo: top half rows = X2T cyclic-shift 1, bottom = shift 65
    XSe: shifts 2 / 66
  XF8 [128, 1152] fp8e4: [X2T(256)|XSo(320) | X2T(256)|XSe(320)] - for the
    DoubleRow S-term matmuls.

Device per iteration it (c0 = 4*it), quarters q0..q3 of 256 pairs each
(q0 = offsets 4it+1|+65, q1 = 4it+3|+67, q2 = 4it+2|+66, q3 = 4it+4|+68):
  A2 [128, 1024] bf16 = |XU - XS[c0 window]|: one fused tensor_tensor
    subtract (on the DVE for even iterations, on GPSIMD for odd ones --
    walrus accepts TensorTensor on Pool, not TensorScalarPtr) + one DVE
    bitand (u16 4x mode) for the abs.
  P1 [128, 1024] f32 psum:
    bf16 matmul  P1 += WBSstack @ A2            (128-contract, 2x512 cols)
    fp8 DoubleRow P1 += WAS@X2T + WAS@XS-window (4 mms, 256 cols @ 2x rate)
  H1 = Lrelu(P1 + b1)  [128, 1024] bf16   (ACT, bias=B1S)
  P2: layer-2 outputs of iteration PAIRS share one [128, 1024]-capable psum
    tile; per iteration 4 bf16 matmuls place each quarter's 16-dim output at
    psum rows 32q (top) / 32q+16 (bottom) -> all 128 rows used.
  act2 = Lrelu(P2 + b2) summed over iteration pairs: 2 of the 8 pair
    groups on the ACT engine (activation + accum_out), 6 on the DVE
    (tensor_scalar + scalar_tensor_tensor max(x, 0.01x) with accum_out)
    so that ACT/DVE/PE/Pool all sit near 79-86% busy.

Layer-2 weights stay bf16: quantizing w2 to fp8 injects noise into pre2,
and E[lrelu(x + noise)] > lrelu(x) (convexity) -> a systematic positive
output bias (~+1.2 measured). fp8 on the layer-1 S-term is bias-free.

Software-pipelined emission with 3-iteration skew: layer-2 of iteration
k-3 is emitted after layer-1 of iteration k so the PE queue never stalls
on act1.

Final combine on host:
  out = x1 + scale*(w3tile . ACC - 0.5*s128_exact + b3*32640).
"""
import numpy as np

L, D = 256, 64
B_PER_CORE = 4
N_CORES = 8
NPAIRS = 32640  # 256*255/2

_CACHE = {}
import os as _os
_os  # env knobs read in _build_program too
N_ITERS = int(_os.environ.get("EPINN_ITERS", "16"))
N_BATCH = int(_os.environ.get("EPINN_BATCH", str(B_PER_CORE)))
N_RUN_CORES = int(_os.environ.get("EPINN_CORES", str(N_CORES)))
USE_FP8 = _os.environ.get("EPINN_FP8", "1") == "1"
USE_FP8_L2 = _os.environ.get("EPINN_FP8_L2", "0") == "1"
N_DVE_ACT2 = int(_os.environ.get("EPINN_DVE_ACT2", "3"))
_ACT_PAIR_SETS = {
    0: (),
    1: (3,),
    2: (1, 5),
    3: (0, 3, 6),
    4: (0, 2, 4, 6),
    5: (0, 1, 3, 5, 6),
    7: (0, 1, 2, 3, 4, 5, 6),
}
ACT_PAIRS = _ACT_PAIR_SETS[int(_os.environ.get("EPINN_ACT_PAIRS", "1"))]
# 0 = A2 subtract always on DVE, 2 = subtract on Pool for odd iterations
# (walrus accepts TensorTensor on Pool but rejects TensorScalarPtr there)
POOL_SUB = int(_os.environ.get("EPINN_POOL_SUB", "2"))
ACT_SOLOS = int(_os.environ.get("EPINN_ACT_SOLOS", "2"))
SKEW = int(_os.environ.get("EPINN_SKEW", "3"))


def _build_program():
    import concourse.bacc as bacc
    import concourse.mybir as mybir
    import concourse.tile as tile
    from concourse.ap import AP
    from contextlib import ExitStack

    f32 = mybir.dt.float32
    bf16 = mybir.dt.bfloat16
    fp8 = mybir.dt.float8e4
    u16 = mybir.dt.uint16
    AF = mybir.ActivationFunctionType
    ALU = mybir.AluOpType
    DR = mybir.MatmulPerfMode.DoubleRow

    nc = bacc.Bacc("TRN2", target_bir_lowering=False, debug=False,
                   num_devices=N_CORES)

    # ---- DRAM parameters (per core) ----
    xall_d = nc.declare_dram_parameter("xall", [N_BATCH, 128, 1152], bf16,
                                       isOutput=False)
    xf8_d = nc.declare_dram_parameter("xf8", [N_BATCH, 128, 1152], fp8,
                                      isOutput=False)
    wk_d = nc.declare_dram_parameter("wk", [128, 1168], bf16, isOutput=False)

    # acc columns: one per iteration pair. The double-counted o=128 stream
    # (iteration 15) no longer needs a separable column: the host recomputes
    # that term exactly and subtracts it.
    assert N_ITERS == 16
    GROUPS = [(0, 1), (2, 3), (4, 5), (6, 7), (8, 9), (10, 11), (12, 13),
              (14, 15)]
    n_acc = len(GROUPS)
    acc_o = nc.declare_dram_parameter("acc_o", [N_BATCH, 128, n_acc], f32,
                                      isOutput=True)

    with tile.TileContext(nc) as tc, ExitStack() as ctx:
        cpool = ctx.enter_context(tc.tile_pool(name="consts", bufs=1))
        XBUFS = int(_os.environ.get("EPINN_XBUFS", "2"))
        xpool = ctx.enter_context(tc.tile_pool(name="xbufs", bufs=XBUFS))
        apool = ctx.enter_context(tc.tile_pool(name="abufs", bufs=2 + SKEW))
        hpool = ctx.enter_context(tc.tile_pool(name="hbufs", bufs=2 + SKEW))
        JBUFS = int(_os.environ.get("EPINN_JBUFS", "2"))
        jpool = ctx.enter_context(tc.tile_pool(name="junk", bufs=JBUFS))
        opool = ctx.enter_context(tc.tile_pool(name="outs", bufs=2))
        pp1 = ctx.enter_context(tc.tile_pool(name="p1", bufs=3, space="PSUM"))
        pp2 = ctx.enter_context(tc.tile_pool(name="p2", bufs=2, space="PSUM"))

        DUM = cpool.tile([1, 2], f32)
        nc.gpsimd.memset(DUM[:], 0.0)
        nc.scalar.activation(DUM[:], DUM[:], AF.Lrelu, scale=1.0, alpha=0.01)

        # batch-0 inputs first: the first DVE subtract only needs XALL, so
        # its DMA leads; weights follow (needed ~1.5us later by the first mm).
        XT0 = xpool.tile([128, 1152], bf16, tag="xall", name="xall0")
        nc.sync.dma_start(XT0[:], xall_d[0])
        WK = cpool.tile([128, 1168], bf16)
        nc.sync.dma_start(WK[:], wk_d[:])
        XF0 = xpool.tile([128, 1152], fp8, tag="xf8", name="xf80")
        if USE_FP8:
            nc.sync.dma_start(XF0[:], xf8_d[0])
        WBS = WK[:, 0:128]
        W2S = WK[:, 128:640]
        WASB = WK[:, 640:768]
        WAS8 = WK[:, 768:896].bitcast(fp8)
        BP = WK[:, 896:912].bitcast(f32)
        B1S = BP[:, 0:1]
        B2S = BP[:, 1:2]
        B1S64 = BP[:, 2:3]
        B2S64 = BP[:, 3:4]
        B2S64001 = BP[:, 4:5]
        WAS2 = WAS8.rearrange("p (i m) -> p i m", i=2)
        W2D = WK[:, 912:1168].bitcast(fp8)  # [128, 512] = 2 groups x [2, 128]

        for b in range(N_BATCH):
            if b == 0:
                XALL, XF8 = XT0, XF0
            else:
                XALL = xpool.tile([128, 1152], bf16, tag="xall")
                XF8 = xpool.tile([128, 1152], fp8, tag="xf8")
                nc.sync.dma_start(XALL[:], xall_d[b])
                if USE_FP8:
                    nc.sync.dma_start(XF8[:], xf8_d[b])

            ACC = opool.tile([128, n_acc], f32, tag="acc")

            def sub_in0():
                base = XALL[:, 0:512]
                return AP(base.tensor, base.offset,
                          [[1152, 128], [0, 2], [256, 2], [1, 256]])

            def sub_in1(c0):
                base = XALL[:, 512 + c0:512 + c0 + 1]
                return AP(base.tensor, base.offset,
                          [[1152, 128], [320, 2], [2, 2], [1, 256]])

            def dr_rhs(h, c0, s):
                base = XF8[:, 576 * h:576 * h + 1]
                return AP(base.tensor, base.offset,
                          [[1152, 128], [256 + c0 + 2 * s, 2], [1, 256]])

            def bf_rhs(h, c0, s):
                # fallback (no fp8): [X2T-copy | XS-window] via two mms
                base = XALL[:, 0:1]
                xu = AP(base.tensor, base.offset, [[1152, 128], [1, 256]])
                b2 = XALL[:, 512 + 320 * h + c0 + 2 * s:]
                xs = AP(b2.tensor, b2.offset, [[1152, 128], [1, 256]])
                return xu, xs

            git = {it: (gi, j, len(g))
                   for gi, g in enumerate(GROUPS) for j, it in enumerate(g)}
            p2_state = [None]  # current group psum tile

            def emit_l2(it, H1):
                gi, j, glen = git[it]
                if j == 0:
                    p2_state[0] = pp2.tile([128, 512], f32, tag="p2",
                                           name="p2t")
                cur = p2_state[0]
                lo = 256 * j
                if USE_FP8_L2:
                    for g in range(2):
                        w2d = W2D[:, 256 * g:256 * g + 256].rearrange(
                            "p (i m) -> p i m", i=2)
                        nc.tensor.matmul(cur[:, lo:lo + 256], w2d,
                                         H1[:, 512 * g:512 * g + 512].rearrange(
                                             "p (i c) -> p i c", i=2),
                                         start=(g == 0), stop=(g == 1),
                                         perf_mode=DR, skip_group_check=True)
                else:
                    for q in range(4):
                        nc.tensor.matmul(cur[:, lo:lo + 256],
                                         W2S[:, 128 * q:128 * q + 128],
                                         H1[:, 256 * q:256 * q + 256],
                                         start=(q == 0), stop=(q == 3),
                                         skip_group_check=True)
                if j != glen - 1:
                    return
                width = 256 * glen
                HJ = jpool.tile([128, 512], bf16, tag="hj")
                # spread most pair-activations onto the DVE (fused
                # lrelu+accumulate) to unload the ACT engine; the rest use
                # ACT with its built-in accumulator.
                # in fp8 mode P2 holds 64*(pre2 - b2); compute 64*lrelu2
                # (homogeneity) and divide acc by 64 on the host.
                on_dve = gi not in ACT_PAIRS and gi != 7
                if on_dve:
                    T2 = jpool.tile([128, 512], bf16, tag="t2")
                    if USE_FP8_L2:
                        nc.vector.tensor_scalar(
                            out=T2[:, 0:width], in0=cur[:, 0:width],
                            scalar1=0.01, scalar2=B2S64001,
                            op0=ALU.mult, op1=ALU.add)
                        nc.vector.scalar_tensor_tensor(
                            out=HJ[:, 0:width], in0=cur[:, 0:width],
                            scalar=B2S64, in1=T2[:, 0:width],
                            op0=ALU.add, op1=ALU.max,
                            accum_out=ACC[:, gi:gi + 1])
                    else:
                        nc.vector.tensor_scalar(
                            out=T2[:, 0:width], in0=cur[:, 0:width],
                            scalar1=B2S, scalar2=0.01,
                            op0=ALU.add, op1=ALU.mult)
                        nc.vector.scalar_tensor_tensor(
                            out=HJ[:, 0:width], in0=cur[:, 0:width],
                            scalar=B2S, in1=T2[:, 0:width],
                            op0=ALU.add, op1=ALU.max,
                            accum_out=ACC[:, gi:gi + 1])
                else:
                    nc.scalar.activation(HJ[:, 0:width], cur[:, 0:width],
                                         AF.Lrelu,
                                         bias=B2S64 if USE_FP8_L2 else B2S,
                                         scale=1.0, alpha=0.01,
                                         accum_out=ACC[:, gi:gi + 1])

            pending = []  # [(it, H1)] pending layer-2 (2-deep skew)
            for it in range(N_ITERS):
                c0 = 4 * it
                A2 = apool.tile([128, 1024], bf16, tag="a2")
                pool_sub = POOL_SUB == 2 and it % 2 == 1
                eng_sub = nc.gpsimd if pool_sub else nc.vector
                eng_sub.tensor_tensor(
                    out=A2[:].rearrange("p (t s c) -> p t s c", t=2, s=2),
                    in0=sub_in0(), in1=sub_in1(c0), op=ALU.subtract)
                nc.vector.tensor_scalar(
                    out=A2[:].bitcast(u16), in0=A2[:].bitcast(u16),
                    scalar1=0x7FFF, scalar2=None, op0=ALU.bitwise_and)

                # ---- layer 1 ----
                P1 = pp1.tile([128, 1024], f32, tag="p1")
                nc.tensor.matmul(P1[:, 0:512], WBS, A2[:, 0:512],
                                 start=True, stop=False, skip_group_check=True)
                nc.tensor.matmul(P1[:, 512:1024], WBS, A2[:, 512:1024],
                                 start=True, stop=False, skip_group_check=True)
                for h in range(2):
                    for s in range(2):
                        pslice = P1[:, 512 * h + 256 * s:512 * h + 256 * s + 256]
                        if USE_FP8:
                            nc.tensor.matmul(pslice, WAS2, dr_rhs(h, c0, s),
                                             start=False, stop=True,
                                             perf_mode=DR,
                                             skip_group_check=True)
                        else:
                            xu, xs = bf_rhs(h, c0, s)
                            nc.tensor.matmul(pslice, WASB, xu,
                                             start=False, stop=False,
                                             skip_group_check=True)
                            nc.tensor.matmul(pslice, WASB, xs,
                                             start=False, stop=True,
                                             skip_group_check=True)

                # software pipeline: layer 2 of iteration it-2 sits behind
                # this iteration's layer 1 in the PE queue.
                if len(pending) >= SKEW:
                    emit_l2(*pending.pop(0))

                H1 = hpool.tile([128, 1024], fp8 if USE_FP8_L2 else bf16,
                                tag="h1")
                if USE_FP8_L2:
                    nc.scalar.activation(H1[:], P1[:], AF.Lrelu, bias=B1S64,
                                         scale=64.0, alpha=0.01)
                else:
                    nc.scalar.activation(H1[:], P1[:], AF.Lrelu, bias=B1S,
                                         scale=1.0, alpha=0.01)
                pending.append((it, H1))

            for p in pending:
                emit_l2(*p)
            nc.sync.dma_start(acc_o[b], ACC[:])

    nc.compile()
    return nc


def _get_program():
    key = (N_ITERS, N_BATCH, USE_FP8, USE_FP8_L2, N_DVE_ACT2, SKEW,
           ACT_PAIRS, POOL_SUB, ACT_SOLOS,
           _os.environ.get("EPINN_JBUFS", "2"),
           _os.environ.get("EPINN_XBUFS", "2"))
    if key not in _CACHE:
        _CACHE[key] = _build_program()
    return _CACHE[key]


def _get_runner():
    """Build (once) a cached jitted SPMD executable for the program."""
    key = ("runner", N_ITERS, N_BATCH, N_RUN_CORES, USE_FP8, USE_FP8_L2,
           N_DVE_ACT2, ACT_PAIRS, POOL_SUB, ACT_SOLOS)
    if key in _CACHE:
        return _CACHE[key]
    import jax
    import jax.numpy as jnp
    import numpy as _np
    import concourse.mybir as mybir
    from jax.sharding import Mesh, PartitionSpec
    from jax.experimental.shard_map import shard_map
    from concourse import bass2jax
    from concourse.bass2jax import _bass_exec_p, partition_id_tensor

    bass2jax.install_neuronx_cc_hook()
    nc = _get_program()
    n_cores = N_RUN_CORES

    partition_name = (nc.partition_id_tensor.name
                      if nc.partition_id_tensor else None)
    in_names, out_names, out_avals, zero_shapes = [], [], [], []
    for alloc in nc.m.functions[0].allocations:
        if not isinstance(alloc, mybir.MemoryLocationSet):
            continue
        name = alloc.memorylocations[0].name
        if alloc.kind == "ExternalInput":
            if name != partition_name:
                in_names.append(name)
        elif alloc.kind == "ExternalOutput":
            out_names.append(name)
            shape = tuple(alloc.tensor_shape)
            dtype = mybir.dt.np(alloc.dtype)
            out_avals.append(jax.core.ShapedArray(shape, dtype))
            zero_shapes.append((shape, dtype))
    n_params = len(in_names)
    n_outs = len(out_avals)
    all_in_names = list(in_names) + list(out_names)
    if partition_name is not None:
        all_in_names.append(partition_name)
    donate = tuple(range(n_params, n_params + n_outs))

    def _body(*args):
        operands = list(args)
        if partition_name is not None:
            operands.append(partition_id_tensor())
        outs = _bass_exec_p.bind(
            *operands, out_avals=tuple(out_avals), in_names=tuple(all_in_names),
            out_names=tuple(out_names), lowering_input_output_aliases=(),
            sim_require_finite=True, sim_require_nnan=True, nc=nc)
        return tuple(outs)

    devices = jax.devices()[:n_cores]
    mesh = Mesh(_np.asarray(devices), ("core",))
    in_specs = (PartitionSpec("core"),) * (n_params + n_outs)
    out_specs = (PartitionSpec("core"),) * len(out_names)
    sharded = jax.jit(
        shard_map(_body, mesh=mesh, in_specs=in_specs, out_specs=out_specs,
                  check_rep=False),
        donate_argnums=donate, keep_unused=True)

    def run(in_maps):
        concat_in = [
            np.concatenate([np.asarray(in_maps[c][nm]) for c in range(n_cores)],
                           axis=0)
            for nm in in_names
        ]
        concat_zeros = [np.zeros((n_cores * s[0], *s[1:]), d)
                        for (s, d) in zero_shapes]
        out_arrs = sharded(*concat_in, *concat_zeros)
        return [
            {nm: np.asarray(out_arrs[i]).reshape(n_cores, *out_avals[i].shape)[c]
             for i, nm in enumerate(out_names)}
            for c in range(n_cores)
        ]

    _CACHE[key] = run
    return run


def _prep_in_maps(emb, tw, w1, b1, w2, b2):
    import ml_dtypes
    bfl = ml_dtypes.bfloat16
    f8 = ml_dtypes.float8_e4m3

    emb = np.asarray(emb, np.float32)
    tw = np.asarray(tw, np.float32)
    w1 = np.asarray(w1, np.float32)
    b1v = np.asarray(b1, np.float32)
    w2f = np.asarray(w2, np.float32)
    b2v = np.asarray(b2, np.float32)

    w1bt = np.ascontiguousarray(w1[:, 64:].T)          # [64f, 64d]
    w1at = np.ascontiguousarray(0.5 * w1[:, :64].T)    # [64f, 64d]
    wbs = np.zeros((128, 128), np.float32)
    wbs[0:64, 0:64] = w1bt
    wbs[64:128, 64:128] = w1bt
    was = np.zeros((128, 128), np.float32)
    was[0:64, 0:64] = w1at
    was[64:128, 64:128] = w1at
    w2s = np.zeros((128, 512), np.float32)
    for q in range(4):
        w2s[0:64, 128 * q + 32 * q:128 * q + 32 * q + 16] = w2f.T
        w2s[64:128, 128 * q + 32 * q + 16:128 * q + 32 * q + 32] = w2f.T
    wp = np.concatenate([wbs, w2s, was], axis=1)       # [128, 768]
    was8 = np.concatenate([was, was], axis=1)          # [128, 256]
    b1d = np.concatenate([b1v, b1v])
    b2d = np.tile(b2v, 8)
    bp = np.stack([b1d, b2d, 64.0 * b1d, 64.0 * b2d, 0.64 * b2d,
                   np.zeros(128, np.float32), np.zeros(128, np.float32),
                   np.zeros(128, np.float32)], axis=1)  # [128, 8]
    w2d = np.zeros((128, 2, 2, 128), np.float32)       # [p, g, i, m]
    for q in range(4):
        w2d[:, q // 2, q % 2, :] = w2s[:, 128 * q:128 * q + 128]
    w2d = w2d.reshape(128, 512)
    wk = np.zeros((128, 1168 * 2), np.uint8)
    wk[:, 0:1536] = wp.astype(bfl).view(np.uint8)
    wk[:, 1536:1792] = was8.astype(f8).view(np.uint8)
    wk[:, 1792:1824] = bp.astype(np.float32).view(np.uint8)
    wk[:, 1824:2336] = w2d.astype(f8).view(np.uint8)
    wk = wk.view(bfl)                                  # [128, 1168]

    twp = tw[:-1].reshape(L, D)
    idx1 = (np.arange(320) + 1) % 256
    idx65 = (np.arange(320) + 65) % 256
    idx2 = (np.arange(320) + 2) % 256
    idx66 = (np.arange(320) + 66) % 256

    shared = {"wk": wk}
    in_maps = []
    for c in range(N_CORES):
        xall = np.zeros((N_BATCH, 128, 1152), np.float32)
        for b in range(N_BATCH):
            gb = c * B_PER_CORE + b
            x2t = (emb[gb, :L * D].reshape(L, D) * twp).T  # [64, 256]
            x2t2 = np.concatenate([x2t, x2t], axis=0)      # [128, 256]
            xall[b, :, 0:256] = x2t2
            xall[b, :, 256:512] = x2t2
            xall[b, 0:64, 512:832] = x2t[:, idx1]
            xall[b, 64:128, 512:832] = x2t[:, idx65]
            xall[b, 0:64, 832:1152] = x2t[:, idx2]
            xall[b, 64:128, 832:1152] = x2t[:, idx66]
        xf8 = np.zeros((N_BATCH, 128, 1152), np.float32)
        xf8[:, :, 0:256] = xall[:, :, 0:256]
        xf8[:, :, 256:576] = xall[:, :, 512:832]    # XSo
        xf8[:, :, 576:832] = xall[:, :, 0:256]      # X2T again
        xf8[:, :, 832:1152] = xall[:, :, 832:1152]  # XSe
        m = dict(shared)
        m["xall"] = xall.astype(bfl)
        m["xf8"] = xf8.astype(f8)
        in_maps.append(m)
    return in_maps


_FP8_SCALED = USE_FP8_L2


def _s128_exact(emb, tw, w1, b1, w2, b2, w3):
    """Exact per-batch sum of s over the 256 ordered (i, i+128) pairs
    (w3 . lrelu2, no b3)."""
    emb = np.asarray(emb, np.float64)
    tw = np.asarray(tw, np.float64)
    x2 = emb[:, :L * D].reshape(-1, L, D) * tw[:-1].reshape(L, D)  # [B,L,D]
    xj = np.roll(x2, -128, axis=1)
    feat = np.concatenate([0.5 * (x2 + xj), np.abs(x2 - xj)], axis=-1)
    pre1 = feat @ np.asarray(w1, np.float64).T + np.asarray(b1, np.float64)
    h1 = np.where(pre1 > 0, pre1, 0.01 * pre1)
    pre2 = h1 @ np.asarray(w2, np.float64).T + np.asarray(b2, np.float64)
    h2 = np.where(pre2 > 0, pre2, 0.01 * pre2)
    return (h2 @ np.asarray(w3, np.float64)[0]).sum(axis=1)  # [B]


def _finish(core_results, emb, tw, tb, w1, b1, w2, b2, w3, b3, scale):
    emb = np.asarray(emb, np.float64)
    tw = np.asarray(tw, np.float64)
    x1 = emb @ tw + float(tb[0])  # [32]
    w3v = np.asarray(w3, np.float32)[0]       # [16]
    w3t = np.tile(w3v, 8)                     # [128]
    s128 = _s128_exact(emb, tw, w1, b1, w2, b2, w3)  # [32]
    out = np.zeros(32, np.float32)
    for c in range(len(core_results)):
        acc = core_results[c]["acc_o"]        # [4, 128, n_acc]
        for b in range(N_BATCH):
            a = acc[b] / 64.0 if _FP8_SCALED else acc[b]
            tot = float(w3t @ a.sum(axis=1))
            gb = c * B_PER_CORE + b
            # o=128 pairs were enumerated twice; remove one copy exactly
            tot -= 0.5 * float(s128[gb])
            out[gb] = (x1[gb]
                       + float(scale[0]) * (tot + float(b3[0]) * NPAIRS))
    return out


def kernel(emb, tw, tb, w1, b1, w2, b2, w3, b3, scale):
    run = _get_runner()
    in_maps = _prep_in_maps(emb, tw, w1, b1, w2, b2)
    core_results = run(in_maps[:N_RUN_CORES])
    return _finish(core_results, emb, tw, tb, w1, b1, w2, b2, w3, b3, scale)


# revision 40
# speedup vs baseline: 1.0138x; 1.0053x over previous
"""Trainium2 Bass kernel for nn_EpiNN_aaindex (pairwise-MLP GNN reduction).

Math (per batch b):
  x1 = emb@tw + tb                               (host)
  X[i,d] = emb[i*64+d] * tw[i*64+d]              (L=256, D=64; host)
  s_ij = MLP(concat[(x_i+x_j)/2, |x_i-x_j|])     (64->16->1, LeakyReLU 0.01)
  out_b = x1 + scale * sum_{i<j} s_ij

Strategy: 8 cores, 4 batches/core (data parallel over B=32).
Exact upper-triangle enumeration via cyclic offsets o=1..128:
pairs (i, (i+o) mod 256) for o=1..127 cover each unordered pair once;
o=128 covers each of its 128 pairs twice -- the host recomputes that one
offset's 256-pair term exactly in numpy and subtracts half of it.

Host prep per batch: X2T = (emb[:, :-1].reshape(L, D) * tw2d).T  [64, 256]
  XALL [128, 1152] bf16: [X2T|X2T (512) | XSo (320) | XSe (320)]
    XSo: top half rows = X2T cyclic-shift 1, bottom = shift 65
    XSe: shifts 2 / 66
  XF8 [128, 1152] fp8e4: [X2T(256)|XSo(320) | X2T(256)|XSe(320)] - for the
    DoubleRow S-term matmuls.

Device per iteration it (c0 = 4*it), quarters q0..q3 of 256 pairs each
(q0 = offsets 4it+1|+65, q1 = 4it+3|+67, q2 = 4it+2|+66, q3 = 4it+4|+68):
  A2 [128, 1024] bf16 = |XU - XS[c0 window]|: one fused tensor_tensor
    subtract (on the DVE for even iterations, on GPSIMD for odd ones --
    walrus accepts TensorTensor on Pool, not TensorScalarPtr) + one DVE
    bitand (u16 4x mode) for the abs.
  P1 [128, 1024] f32 psum:
    bf16 matmul  P1 += WBSstack @ A2            (128-contract, 2x512 cols)
    fp8 DoubleRow P1 += WAS@X2T + WAS@XS-window (4 mms, 256 cols @ 2x rate)
  H1 = Lrelu(P1 + b1)  [128, 1024] bf16   (ACT, bias=B1S)
  P2: layer-2 outputs of iteration PAIRS share one [128, 1024]-capable psum
    tile; per iteration 4 bf16 matmuls place each quarter's 16-dim output at
    psum rows 32q (top) / 32q+16 (bottom) -> all 128 rows used.
  act2 = Lrelu(P2 + b2) summed over iteration pairs: 2 of the 8 pair
    groups on the ACT engine (activation + accum_out), 6 on the DVE
    (tensor_scalar + scalar_tensor_tensor max(x, 0.01x) with accum_out)
    so that ACT/DVE/PE/Pool all sit near 79-86% busy.

Layer-2 weights stay bf16: quantizing w2 to fp8 injects noise into pre2,
and E[lrelu(x + noise)] > lrelu(x) (convexity) -> a systematic positive
output bias (~+1.2 measured). fp8 on the layer-1 S-term is bias-free.

Software-pipelined emission with 3-iteration skew: layer-2 of iteration
k-3 is emitted after layer-1 of iteration k so the PE queue never stalls
on act1.

Final combine on host:
  out = x1 + scale*(w3tile . ACC - 0.5*s128_exact + b3*32640).
"""
import numpy as np

L, D = 256, 64
B_PER_CORE = 4
N_CORES = 8
NPAIRS = 32640  # 256*255/2

_CACHE = {}
import os as _os
_os  # env knobs read in _build_program too
N_ITERS = int(_os.environ.get("EPINN_ITERS", "16"))
N_BATCH = int(_os.environ.get("EPINN_BATCH", str(B_PER_CORE)))
N_RUN_CORES = int(_os.environ.get("EPINN_CORES", str(N_CORES)))
USE_FP8 = _os.environ.get("EPINN_FP8", "1") == "1"
USE_FP8_L2 = _os.environ.get("EPINN_FP8_L2", "0") == "1"
N_DVE_ACT2 = int(_os.environ.get("EPINN_DVE_ACT2", "3"))
_ACT_PAIR_SETS = {
    0: (),
    1: (3,),
    2: (1, 5),
    3: (0, 3, 6),
    4: (0, 2, 4, 6),
    5: (0, 1, 3, 5, 6),
    7: (0, 1, 2, 3, 4, 5, 6),
    10: (0,), 11: (1,), 12: (2,), 14: (4,), 15: (5,), 16: (6,),
}
ACT_PAIRS = _ACT_PAIR_SETS[int(_os.environ.get("EPINN_ACT_PAIRS", "14"))]
# 0 = A2 subtract always on DVE, 2 = subtract on Pool for odd iterations
# (walrus accepts TensorTensor on Pool but rejects TensorScalarPtr there)
POOL_SUB = int(_os.environ.get("EPINN_POOL_SUB", "2"))
ACT_SOLOS = int(_os.environ.get("EPINN_ACT_SOLOS", "2"))
SKEW = int(_os.environ.get("EPINN_SKEW", "3"))


def _build_program():
    import concourse.bacc as bacc
    import concourse.mybir as mybir
    import concourse.tile as tile
    from concourse.ap import AP
    from contextlib import ExitStack

    f32 = mybir.dt.float32
    bf16 = mybir.dt.bfloat16
    fp8 = mybir.dt.float8e4
    u16 = mybir.dt.uint16
    AF = mybir.ActivationFunctionType
    ALU = mybir.AluOpType
    DR = mybir.MatmulPerfMode.DoubleRow

    nc = bacc.Bacc("TRN2", target_bir_lowering=False, debug=False,
                   num_devices=N_CORES)

    # ---- DRAM parameters (per core) ----
    xall_d = nc.declare_dram_parameter("xall", [N_BATCH, 128, 1152], bf16,
                                       isOutput=False)
    xf8_d = nc.declare_dram_parameter("xf8", [N_BATCH, 128, 1152], fp8,
                                      isOutput=False)
    wk_d = nc.declare_dram_parameter("wk", [128, 1168], bf16, isOutput=False)

    # acc columns: one per iteration pair. The double-counted o=128 stream
    # (iteration 15) no longer needs a separable column: the host recomputes
    # that term exactly and subtracts it.
    assert N_ITERS == 16
    GROUPS = [(0, 1), (2, 3), (4, 5), (6, 7), (8, 9), (10, 11), (12, 13),
              (14, 15)]
    n_acc = len(GROUPS)
    acc_o = nc.declare_dram_parameter("acc_o", [N_BATCH, 128, n_acc], f32,
                                      isOutput=True)

    with tile.TileContext(nc) as tc, ExitStack() as ctx:
        cpool = ctx.enter_context(tc.tile_pool(name="consts", bufs=1))
        XBUFS = int(_os.environ.get("EPINN_XBUFS", "2"))
        xpool = ctx.enter_context(tc.tile_pool(name="xbufs", bufs=XBUFS))
        apool = ctx.enter_context(tc.tile_pool(name="abufs", bufs=2 + SKEW))
        hpool = ctx.enter_context(tc.tile_pool(name="hbufs", bufs=2 + SKEW))
        JBUFS = int(_os.environ.get("EPINN_JBUFS", "2"))
        jpool = ctx.enter_context(tc.tile_pool(name="junk", bufs=JBUFS))
        opool = ctx.enter_context(tc.tile_pool(name="outs", bufs=2))
        pp1 = ctx.enter_context(tc.tile_pool(name="p1", bufs=3, space="PSUM"))
        pp2 = ctx.enter_context(tc.tile_pool(name="p2", bufs=2, space="PSUM"))

        DUM = cpool.tile([1, 2], f32)
        nc.gpsimd.memset(DUM[:], 0.0)
        nc.scalar.activation(DUM[:], DUM[:], AF.Lrelu, scale=1.0, alpha=0.01)

        # batch-0 inputs first: the first DVE subtract only needs XALL, so
        # its DMA leads; weights follow (needed ~1.5us later by the first mm).
        XT0 = xpool.tile([128, 1152], bf16, tag="xall", name="xall0")
        nc.sync.dma_start(XT0[:], xall_d[0])
        WK = cpool.tile([128, 1168], bf16)
        nc.sync.dma_start(WK[:], wk_d[:])
        XF0 = xpool.tile([128, 1152], fp8, tag="xf8", name="xf80")
        if USE_FP8:
            nc.sync.dma_start(XF0[:], xf8_d[0])
        WBS = WK[:, 0:128]
        W2S = WK[:, 128:640]
        WASB = WK[:, 640:768]
        WAS8 = WK[:, 768:896].bitcast(fp8)
        BP = WK[:, 896:912].bitcast(f32)
        B1S = BP[:, 0:1]
        B2S = BP[:, 1:2]
        B1S64 = BP[:, 2:3]
        B2S64 = BP[:, 3:4]
        B2S64001 = BP[:, 4:5]
        WAS2 = WAS8.rearrange("p (i m) -> p i m", i=2)
        W2D = WK[:, 912:1168].bitcast(fp8)  # [128, 512] = 2 groups x [2, 128]

        for b in range(N_BATCH):
            if b == 0:
                XALL, XF8 = XT0, XF0
            else:
                XALL = xpool.tile([128, 1152], bf16, tag="xall")
                XF8 = xpool.tile([128, 1152], fp8, tag="xf8")
                nc.sync.dma_start(XALL[:], xall_d[b])
                if USE_FP8:
                    nc.sync.dma_start(XF8[:], xf8_d[b])

            ACC = opool.tile([128, n_acc], f32, tag="acc")

            def sub_in0():
                base = XALL[:, 0:512]
                return AP(base.tensor, base.offset,
                          [[1152, 128], [0, 2], [256, 2], [1, 256]])

            def sub_in1(c0):
                base = XALL[:, 512 + c0:512 + c0 + 1]
                return AP(base.tensor, base.offset,
                          [[1152, 128], [320, 2], [2, 2], [1, 256]])

            def dr_rhs(h, c0, s):
                base = XF8[:, 576 * h:576 * h + 1]
                return AP(base.tensor, base.offset,
                          [[1152, 128], [256 + c0 + 2 * s, 2], [1, 256]])

            def bf_rhs(h, c0, s):
                # fallback (no fp8): [X2T-copy | XS-window] via two mms
                base = XALL[:, 0:1]
                xu = AP(base.tensor, base.offset, [[1152, 128], [1, 256]])
                b2 = XALL[:, 512 + 320 * h + c0 + 2 * s:]
                xs = AP(b2.tensor, b2.offset, [[1152, 128], [1, 256]])
                return xu, xs

            git = {it: (gi, j, len(g))
                   for gi, g in enumerate(GROUPS) for j, it in enumerate(g)}
            p2_state = [None]  # current group psum tile

            def emit_l2(it, H1):
                gi, j, glen = git[it]
                if j == 0:
                    p2_state[0] = pp2.tile([128, 512], f32, tag="p2",
                                           name="p2t")
                cur = p2_state[0]
                lo = 256 * j
                if USE_FP8_L2:
                    for g in range(2):
                        w2d = W2D[:, 256 * g:256 * g + 256].rearrange(
                            "p (i m) -> p i m", i=2)
                        nc.tensor.matmul(cur[:, lo:lo + 256], w2d,
                                         H1[:, 512 * g:512 * g + 512].rearrange(
                                             "p (i c) -> p i c", i=2),
                                         start=(g == 0), stop=(g == 1),
                                         perf_mode=DR, skip_group_check=True)
                else:
                    for q in range(4):
                        nc.tensor.matmul(cur[:, lo:lo + 256],
                                         W2S[:, 128 * q:128 * q + 128],
                                         H1[:, 256 * q:256 * q + 256],
                                         start=(q == 0), stop=(q == 3),
                                         skip_group_check=True)
                if j != glen - 1:
                    return
                width = 256 * glen
                HJ = jpool.tile([128, 512], bf16, tag="hj")
                # spread most pair-activations onto the DVE (fused
                # lrelu+accumulate) to unload the ACT engine; the rest use
                # ACT with its built-in accumulator.
                # in fp8 mode P2 holds 64*(pre2 - b2); compute 64*lrelu2
                # (homogeneity) and divide acc by 64 on the host.
                on_dve = gi not in ACT_PAIRS and gi != 7
                if on_dve:
                    T2 = jpool.tile([128, 512], bf16, tag="t2")
                    if USE_FP8_L2:
                        nc.vector.tensor_scalar(
                            out=T2[:, 0:width], in0=cur[:, 0:width],
                            scalar1=0.01, scalar2=B2S64001,
                            op0=ALU.mult, op1=ALU.add)
                        nc.vector.scalar_tensor_tensor(
                            out=HJ[:, 0:width], in0=cur[:, 0:width],
                            scalar=B2S64, in1=T2[:, 0:width],
                            op0=ALU.add, op1=ALU.max,
                            accum_out=ACC[:, gi:gi + 1])
                    else:
                        nc.vector.tensor_scalar(
                            out=T2[:, 0:width], in0=cur[:, 0:width],
                            scalar1=B2S, scalar2=0.01,
                            op0=ALU.add, op1=ALU.mult)
                        nc.vector.scalar_tensor_tensor(
                            out=HJ[:, 0:width], in0=cur[:, 0:width],
                            scalar=B2S, in1=T2[:, 0:width],
                            op0=ALU.add, op1=ALU.max,
                            accum_out=ACC[:, gi:gi + 1])
                else:
                    nc.scalar.activation(HJ[:, 0:width], cur[:, 0:width],
                                         AF.Lrelu,
                                         bias=B2S64 if USE_FP8_L2 else B2S,
                                         scale=1.0, alpha=0.01,
                                         accum_out=ACC[:, gi:gi + 1])

            pending = []  # [(it, H1)] pending layer-2 (2-deep skew)
            for it in range(N_ITERS):
                c0 = 4 * it
                A2 = apool.tile([128, 1024], bf16, tag="a2")
                pool_sub = POOL_SUB == 2 and it % 2 == 1
                eng_sub = nc.gpsimd if pool_sub else nc.vector
                eng_sub.tensor_tensor(
                    out=A2[:].rearrange("p (t s c) -> p t s c", t=2, s=2),
                    in0=sub_in0(), in1=sub_in1(c0), op=ALU.subtract)
                nc.vector.tensor_scalar(
                    out=A2[:].bitcast(u16), in0=A2[:].bitcast(u16),
                    scalar1=0x7FFF, scalar2=None, op0=ALU.bitwise_and)

                # ---- layer 1 ----
                P1 = pp1.tile([128, 1024], f32, tag="p1")
                nc.tensor.matmul(P1[:, 0:512], WBS, A2[:, 0:512],
                                 start=True, stop=False, skip_group_check=True)
                nc.tensor.matmul(P1[:, 512:1024], WBS, A2[:, 512:1024],
                                 start=True, stop=False, skip_group_check=True)
                for h in range(2):
                    for s in range(2):
                        pslice = P1[:, 512 * h + 256 * s:512 * h + 256 * s + 256]
                        if USE_FP8:
                            nc.tensor.matmul(pslice, WAS2, dr_rhs(h, c0, s),
                                             start=False, stop=True,
                                             perf_mode=DR,
                                             skip_group_check=True)
                        else:
                            xu, xs = bf_rhs(h, c0, s)
                            nc.tensor.matmul(pslice, WASB, xu,
                                             start=False, stop=False,
                                             skip_group_check=True)
                            nc.tensor.matmul(pslice, WASB, xs,
                                             start=False, stop=True,
                                             skip_group_check=True)

                # software pipeline: layer 2 of iteration it-2 sits behind
                # this iteration's layer 1 in the PE queue.
                if len(pending) >= SKEW:
                    emit_l2(*pending.pop(0))

                H1 = hpool.tile([128, 1024], fp8 if USE_FP8_L2 else bf16,
                                tag="h1")
                if USE_FP8_L2:
                    nc.scalar.activation(H1[:], P1[:], AF.Lrelu, bias=B1S64,
                                         scale=64.0, alpha=0.01)
                else:
                    nc.scalar.activation(H1[:], P1[:], AF.Lrelu, bias=B1S,
                                         scale=1.0, alpha=0.01)
                pending.append((it, H1))

            for p in pending:
                emit_l2(*p)
            nc.sync.dma_start(acc_o[b], ACC[:])

    nc.compile()
    return nc


def _get_program():
    key = (N_ITERS, N_BATCH, USE_FP8, USE_FP8_L2, N_DVE_ACT2, SKEW,
           ACT_PAIRS, POOL_SUB, ACT_SOLOS,
           _os.environ.get("EPINN_JBUFS", "2"),
           _os.environ.get("EPINN_XBUFS", "2"))
    if key not in _CACHE:
        _CACHE[key] = _build_program()
    return _CACHE[key]


def _get_runner():
    """Build (once) a cached jitted SPMD executable for the program."""
    key = ("runner", N_ITERS, N_BATCH, N_RUN_CORES, USE_FP8, USE_FP8_L2,
           N_DVE_ACT2, ACT_PAIRS, POOL_SUB, ACT_SOLOS)
    if key in _CACHE:
        return _CACHE[key]
    import jax
    import jax.numpy as jnp
    import numpy as _np
    import concourse.mybir as mybir
    from jax.sharding import Mesh, PartitionSpec
    from jax.experimental.shard_map import shard_map
    from concourse import bass2jax
    from concourse.bass2jax import _bass_exec_p, partition_id_tensor

    bass2jax.install_neuronx_cc_hook()
    nc = _get_program()
    n_cores = N_RUN_CORES

    partition_name = (nc.partition_id_tensor.name
                      if nc.partition_id_tensor else None)
    in_names, out_names, out_avals, zero_shapes = [], [], [], []
    for alloc in nc.m.functions[0].allocations:
        if not isinstance(alloc, mybir.MemoryLocationSet):
            continue
        name = alloc.memorylocations[0].name
        if alloc.kind == "ExternalInput":
            if name != partition_name:
                in_names.append(name)
        elif alloc.kind == "ExternalOutput":
            out_names.append(name)
            shape = tuple(alloc.tensor_shape)
            dtype = mybir.dt.np(alloc.dtype)
            out_avals.append(jax.core.ShapedArray(shape, dtype))
            zero_shapes.append((shape, dtype))
    n_params = len(in_names)
    n_outs = len(out_avals)
    all_in_names = list(in_names) + list(out_names)
    if partition_name is not None:
        all_in_names.append(partition_name)
    donate = tuple(range(n_params, n_params + n_outs))

    def _body(*args):
        operands = list(args)
        if partition_name is not None:
            operands.append(partition_id_tensor())
        outs = _bass_exec_p.bind(
            *operands, out_avals=tuple(out_avals), in_names=tuple(all_in_names),
            out_names=tuple(out_names), lowering_input_output_aliases=(),
            sim_require_finite=True, sim_require_nnan=True, nc=nc)
        return tuple(outs)

    devices = jax.devices()[:n_cores]
    mesh = Mesh(_np.asarray(devices), ("core",))
    in_specs = (PartitionSpec("core"),) * (n_params + n_outs)
    out_specs = (PartitionSpec("core"),) * len(out_names)
    sharded = jax.jit(
        shard_map(_body, mesh=mesh, in_specs=in_specs, out_specs=out_specs,
                  check_rep=False),
        donate_argnums=donate, keep_unused=True)

    def run(in_maps):
        concat_in = [
            np.concatenate([np.asarray(in_maps[c][nm]) for c in range(n_cores)],
                           axis=0)
            for nm in in_names
        ]
        concat_zeros = [np.zeros((n_cores * s[0], *s[1:]), d)
                        for (s, d) in zero_shapes]
        out_arrs = sharded(*concat_in, *concat_zeros)
        return [
            {nm: np.asarray(out_arrs[i]).reshape(n_cores, *out_avals[i].shape)[c]
             for i, nm in enumerate(out_names)}
            for c in range(n_cores)
        ]

    _CACHE[key] = run
    return run


def _prep_in_maps(emb, tw, w1, b1, w2, b2):
    import ml_dtypes
    bfl = ml_dtypes.bfloat16
    f8 = ml_dtypes.float8_e4m3

    emb = np.asarray(emb, np.float32)
    tw = np.asarray(tw, np.float32)
    w1 = np.asarray(w1, np.float32)
    b1v = np.asarray(b1, np.float32)
    w2f = np.asarray(w2, np.float32)
    b2v = np.asarray(b2, np.float32)

    w1bt = np.ascontiguousarray(w1[:, 64:].T)          # [64f, 64d]
    w1at = np.ascontiguousarray(0.5 * w1[:, :64].T)    # [64f, 64d]
    wbs = np.zeros((128, 128), np.float32)
    wbs[0:64, 0:64] = w1bt
    wbs[64:128, 64:128] = w1bt
    was = np.zeros((128, 128), np.float32)
    was[0:64, 0:64] = w1at
    was[64:128, 64:128] = w1at
    w2s = np.zeros((128, 512), np.float32)
    for q in range(4):
        w2s[0:64, 128 * q + 32 * q:128 * q + 32 * q + 16] = w2f.T
        w2s[64:128, 128 * q + 32 * q + 16:128 * q + 32 * q + 32] = w2f.T
    wp = np.concatenate([wbs, w2s, was], axis=1)       # [128, 768]
    was8 = np.concatenate([was, was], axis=1)          # [128, 256]
    b1d = np.concatenate([b1v, b1v])
    b2d = np.tile(b2v, 8)
    bp = np.stack([b1d, b2d, 64.0 * b1d, 64.0 * b2d, 0.64 * b2d,
                   np.zeros(128, np.float32), np.zeros(128, np.float32),
                   np.zeros(128, np.float32)], axis=1)  # [128, 8]
    w2d = np.zeros((128, 2, 2, 128), np.float32)       # [p, g, i, m]
    for q in range(4):
        w2d[:, q // 2, q % 2, :] = w2s[:, 128 * q:128 * q + 128]
    w2d = w2d.reshape(128, 512)
    wk = np.zeros((128, 1168 * 2), np.uint8)
    wk[:, 0:1536] = wp.astype(bfl).view(np.uint8)
    wk[:, 1536:1792] = was8.astype(f8).view(np.uint8)
    wk[:, 1792:1824] = bp.astype(np.float32).view(np.uint8)
    wk[:, 1824:2336] = w2d.astype(f8).view(np.uint8)
    wk = wk.view(bfl)                                  # [128, 1168]

    twp = tw[:-1].reshape(L, D)
    idx1 = (np.arange(320) + 1) % 256
    idx65 = (np.arange(320) + 65) % 256
    idx2 = (np.arange(320) + 2) % 256
    idx66 = (np.arange(320) + 66) % 256

    shared = {"wk": wk}
    in_maps = []
    for c in range(N_CORES):
        xall = np.zeros((N_BATCH, 128, 1152), np.float32)
        for b in range(N_BATCH):
            gb = c * B_PER_CORE + b
            x2t = (emb[gb, :L * D].reshape(L, D) * twp).T  # [64, 256]
            x2t2 = np.concatenate([x2t, x2t], axis=0)      # [128, 256]
            xall[b, :, 0:256] = x2t2
            xall[b, :, 256:512] = x2t2
            xall[b, 0:64, 512:832] = x2t[:, idx1]
            xall[b, 64:128, 512:832] = x2t[:, idx65]
            xall[b, 0:64, 832:1152] = x2t[:, idx2]
            xall[b, 64:128, 832:1152] = x2t[:, idx66]
        xf8 = np.zeros((N_BATCH, 128, 1152), np.float32)
        xf8[:, :, 0:256] = xall[:, :, 0:256]
        xf8[:, :, 256:576] = xall[:, :, 512:832]    # XSo
        xf8[:, :, 576:832] = xall[:, :, 0:256]      # X2T again
        xf8[:, :, 832:1152] = xall[:, :, 832:1152]  # XSe
        m = dict(shared)
        m["xall"] = xall.astype(bfl)
        m["xf8"] = xf8.astype(f8)
        in_maps.append(m)
    return in_maps


_FP8_SCALED = USE_FP8_L2


def _s128_exact(emb, tw, w1, b1, w2, b2, w3):
    """Exact per-batch sum of s over the 256 ordered (i, i+128) pairs
    (w3 . lrelu2, no b3)."""
    emb = np.asarray(emb, np.float64)
    tw = np.asarray(tw, np.float64)
    x2 = emb[:, :L * D].reshape(-1, L, D) * tw[:-1].reshape(L, D)  # [B,L,D]
    xj = np.roll(x2, -128, axis=1)
    feat = np.concatenate([0.5 * (x2 + xj), np.abs(x2 - xj)], axis=-1)
    pre1 = feat @ np.asarray(w1, np.float64).T + np.asarray(b1, np.float64)
    h1 = np.where(pre1 > 0, pre1, 0.01 * pre1)
    pre2 = h1 @ np.asarray(w2, np.float64).T + np.asarray(b2, np.float64)
    h2 = np.where(pre2 > 0, pre2, 0.01 * pre2)
    return (h2 @ np.asarray(w3, np.float64)[0]).sum(axis=1)  # [B]


def _finish(core_results, emb, tw, tb, w1, b1, w2, b2, w3, b3, scale):
    emb = np.asarray(emb, np.float64)
    tw = np.asarray(tw, np.float64)
    x1 = emb @ tw + float(tb[0])  # [32]
    w3v = np.asarray(w3, np.float32)[0]       # [16]
    w3t = np.tile(w3v, 8)                     # [128]
    s128 = _s128_exact(emb, tw, w1, b1, w2, b2, w3)  # [32]
    out = np.zeros(32, np.float32)
    for c in range(len(core_results)):
        acc = core_results[c]["acc_o"]        # [4, 128, n_acc]
        for b in range(N_BATCH):
            a = acc[b] / 64.0 if _FP8_SCALED else acc[b]
            tot = float(w3t @ a.sum(axis=1))
            gb = c * B_PER_CORE + b
            # o=128 pairs were enumerated twice; remove one copy exactly
            tot -= 0.5 * float(s128[gb])
            out[gb] = (x1[gb]
                       + float(scale[0]) * (tot + float(b3[0]) * NPAIRS))
    return out


def kernel(emb, tw, tb, w1, b1, w2, b2, w3, b3, scale):
    run = _get_runner()
    in_maps = _prep_in_maps(emb, tw, w1, b1, w2, b2)
    core_results = run(in_maps[:N_RUN_CORES])
    return _finish(core_results, emb, tw, tb, w1, b1, w2, b2, w3, b3, scale)


# revision 41
# speedup vs baseline: 1.0200x; 1.0062x over previous
"""Trainium2 Bass kernel for nn_EpiNN_aaindex (pairwise-MLP GNN reduction).

Math (per batch b):
  x1 = emb@tw + tb                               (host)
  X[i,d] = emb[i*64+d] * tw[i*64+d]              (L=256, D=64; host)
  s_ij = MLP(concat[(x_i+x_j)/2, |x_i-x_j|])     (64->16->1, LeakyReLU 0.01)
  out_b = x1 + scale * sum_{i<j} s_ij

Strategy: 8 cores, 4 batches/core (data parallel over B=32).
Exact upper-triangle enumeration via cyclic offsets o=1..128:
pairs (i, (i+o) mod 256) for o=1..127 cover each unordered pair once;
o=128 covers each of its 128 pairs twice -- the host recomputes that one
offset's 256-pair term exactly in numpy and subtracts half of it.

Host prep per batch: X2T = (emb[:, :-1].reshape(L, D) * tw2d).T  [64, 256]
  XALL [128, 1152] bf16: [X2T|X2T (512) | XSo (320) | XSe (320)]
    XSo: top half rows = X2T cyclic-shift 1, bottom = shift 65
    XSe: shifts 2 / 66
  XF8 [128, 1152] fp8e4: [X2T(256)|XSo(320) | X2T(256)|XSe(320)] - for the
    DoubleRow S-term matmuls.

Device per iteration it (c0 = 4*it), quarters q0..q3 of 256 pairs each
(q0 = offsets 4it+1|+65, q1 = 4it+3|+67, q2 = 4it+2|+66, q3 = 4it+4|+68):
  A2 [128, 1024] bf16 = |XU - XS[c0 window]|: one fused tensor_tensor
    subtract (on the DVE for even iterations, on GPSIMD for odd ones --
    walrus accepts TensorTensor on Pool, not TensorScalarPtr) + one DVE
    bitand (u16 4x mode) for the abs.
  P1 [128, 1024] f32 psum:
    bf16 matmul  P1 += WBSstack @ A2            (128-contract, 2x512 cols)
    fp8 DoubleRow P1 += WAS@X2T + WAS@XS-window (4 mms, 256 cols @ 2x rate)
  H1 = Lrelu(P1 + b1)  [128, 1024] bf16   (ACT, bias=B1S)
  P2: layer-2 outputs of iteration PAIRS share one [128, 1024]-capable psum
    tile; per iteration 4 bf16 matmuls place each quarter's 16-dim output at
    psum rows 32q (top) / 32q+16 (bottom) -> all 128 rows used.
  act2 = Lrelu(P2 + b2) summed over iteration pairs: 2 of the 8 pair
    groups on the ACT engine (activation + accum_out), 6 on the DVE
    (tensor_scalar + scalar_tensor_tensor max(x, 0.01x) with accum_out)
    so that ACT/DVE/PE/Pool all sit near 79-86% busy.

Layer-2 weights stay bf16: quantizing w2 to fp8 injects noise into pre2,
and E[lrelu(x + noise)] > lrelu(x) (convexity) -> a systematic positive
output bias (~+1.2 measured). fp8 on the layer-1 S-term is bias-free.

Software-pipelined emission with 3-iteration skew: layer-2 of iteration
k-3 is emitted after layer-1 of iteration k so the PE queue never stalls
on act1.

Final combine on host:
  out = x1 + scale*(w3tile . ACC - 0.5*s128_exact + b3*32640).
"""
import numpy as np

L, D = 256, 64
B_PER_CORE = 4
N_CORES = 8
NPAIRS = 32640  # 256*255/2

_CACHE = {}
import os as _os
_os  # env knobs read in _build_program too
N_ITERS = int(_os.environ.get("EPINN_ITERS", "16"))
N_BATCH = int(_os.environ.get("EPINN_BATCH", str(B_PER_CORE)))
N_RUN_CORES = int(_os.environ.get("EPINN_CORES", str(N_CORES)))
USE_FP8 = _os.environ.get("EPINN_FP8", "1") == "1"
USE_FP8_L2 = _os.environ.get("EPINN_FP8_L2", "0") == "1"
N_DVE_ACT2 = int(_os.environ.get("EPINN_DVE_ACT2", "3"))
_ACT_PAIR_SETS = {
    0: (),
    1: (3,),
    2: (1, 5),
    3: (0, 3, 6),
    4: (0, 2, 4, 6),
    5: (0, 1, 3, 5, 6),
    7: (0, 1, 2, 3, 4, 5, 6),
    10: (0,), 11: (1,), 12: (2,), 14: (4,), 15: (5,), 16: (6,),
}
ACT_PAIRS = _ACT_PAIR_SETS[int(_os.environ.get("EPINN_ACT_PAIRS", "14"))]
# 0 = A2 subtract always on DVE, 2 = subtract on Pool for odd iterations
# (walrus accepts TensorTensor on Pool but rejects TensorScalarPtr there)
POOL_SUB = int(_os.environ.get("EPINN_POOL_SUB", "2"))
ACT_SOLOS = int(_os.environ.get("EPINN_ACT_SOLOS", "2"))
SKEW = int(_os.environ.get("EPINN_SKEW", "3"))


def _build_program():
    import concourse.bacc as bacc
    import concourse.mybir as mybir
    import concourse.tile as tile
    from concourse.ap import AP
    from contextlib import ExitStack

    f32 = mybir.dt.float32
    bf16 = mybir.dt.bfloat16
    fp8 = mybir.dt.float8e4
    u16 = mybir.dt.uint16
    AF = mybir.ActivationFunctionType
    ALU = mybir.AluOpType
    DR = mybir.MatmulPerfMode.DoubleRow

    nc = bacc.Bacc("TRN2", target_bir_lowering=False, debug=False,
                   num_devices=N_CORES)

    # ---- DRAM parameters (per core) ----
    xall_d = nc.declare_dram_parameter("xall", [N_BATCH, 128, 1152], bf16,
                                       isOutput=False)
    xf8_d = nc.declare_dram_parameter("xf8", [N_BATCH, 128, 1152], fp8,
                                      isOutput=False)
    wk_d = nc.declare_dram_parameter("wk", [128, 1168], bf16, isOutput=False)

    # acc columns: one per iteration pair. The double-counted o=128 stream
    # (iteration 15) no longer needs a separable column: the host recomputes
    # that term exactly and subtracts it.
    assert N_ITERS == 16
    GROUPS = [(0, 1), (2, 3), (4, 5), (6, 7), (8, 9), (10, 11), (12, 13),
              (14, 15)]
    n_acc = len(GROUPS)
    acc_o = nc.declare_dram_parameter("acc_o", [N_BATCH, 128, n_acc], f32,
                                      isOutput=True)

    with tile.TileContext(nc) as tc, ExitStack() as ctx:
        cpool = ctx.enter_context(tc.tile_pool(name="consts", bufs=1))
        XBUFS = int(_os.environ.get("EPINN_XBUFS", "2"))
        xpool = ctx.enter_context(tc.tile_pool(name="xbufs", bufs=XBUFS))
        apool = ctx.enter_context(tc.tile_pool(name="abufs", bufs=2 + SKEW))
        hpool = ctx.enter_context(tc.tile_pool(name="hbufs", bufs=2 + SKEW))
        JBUFS = int(_os.environ.get("EPINN_JBUFS", "2"))
        jpool = ctx.enter_context(tc.tile_pool(name="junk", bufs=JBUFS))
        opool = ctx.enter_context(tc.tile_pool(name="outs", bufs=2))
        pp1 = ctx.enter_context(tc.tile_pool(name="p1", bufs=3, space="PSUM"))
        pp2 = ctx.enter_context(tc.tile_pool(name="p2", bufs=2, space="PSUM"))

        DUM = cpool.tile([1, 2], f32)
        nc.gpsimd.memset(DUM[:], 0.0)
        nc.scalar.activation(DUM[:], DUM[:], AF.Lrelu, scale=1.0, alpha=0.01)

        # batch-0 inputs first: the first DVE subtract only needs XALL, so
        # its DMA leads; weights follow (needed ~1.5us later by the first mm).
        XT0 = xpool.tile([128, 1152], bf16, tag="xall", name="xall0")
        nc.sync.dma_start(XT0[:], xall_d[0])
        WK = cpool.tile([128, 1168], bf16)
        nc.sync.dma_start(WK[:], wk_d[:])
        XF0 = xpool.tile([128, 1152], fp8, tag="xf8", name="xf80")
        if USE_FP8:
            nc.sync.dma_start(XF0[:], xf8_d[0])
        WBS = WK[:, 0:128]
        W2S = WK[:, 128:640]
        WASB = WK[:, 640:768]
        WAS8 = WK[:, 768:896].bitcast(fp8)
        BP = WK[:, 896:912].bitcast(f32)
        B1S = BP[:, 0:1]
        B2S = BP[:, 1:2]
        B1S64 = BP[:, 2:3]
        B2S64 = BP[:, 3:4]
        B2S64001 = BP[:, 4:5]
        WAS2 = WAS8.rearrange("p (i m) -> p i m", i=2)
        W2D = WK[:, 912:1168].bitcast(fp8)  # [128, 512] = 2 groups x [2, 128]

        git = {it: (gi, j, len(g))
               for gi, g in enumerate(GROUPS) for j, it in enumerate(g)}
        p2_state = [None]  # current group psum tile
        pending = []       # [(b, ACC, it, H1)] layer-2 backlog (SKEW deep)

        for b in range(N_BATCH):
            if b == 0:
                XALL, XF8 = XT0, XF0
            else:
                XALL = xpool.tile([128, 1152], bf16, tag="xall")
                XF8 = xpool.tile([128, 1152], fp8, tag="xf8")
                nc.sync.dma_start(XALL[:], xall_d[b])
                if USE_FP8:
                    nc.sync.dma_start(XF8[:], xf8_d[b])

            ACC = opool.tile([128, n_acc], f32, tag="acc")

            def sub_in0():
                base = XALL[:, 0:512]
                return AP(base.tensor, base.offset,
                          [[1152, 128], [0, 2], [256, 2], [1, 256]])

            def sub_in1(c0):
                base = XALL[:, 512 + c0:512 + c0 + 1]
                return AP(base.tensor, base.offset,
                          [[1152, 128], [320, 2], [2, 2], [1, 256]])

            def dr_rhs(h, c0, s):
                base = XF8[:, 576 * h:576 * h + 1]
                return AP(base.tensor, base.offset,
                          [[1152, 128], [256 + c0 + 2 * s, 2], [1, 256]])

            def bf_rhs(h, c0, s):
                # fallback (no fp8): [X2T-copy | XS-window] via two mms
                base = XALL[:, 0:1]
                xu = AP(base.tensor, base.offset, [[1152, 128], [1, 256]])
                b2 = XALL[:, 512 + 320 * h + c0 + 2 * s:]
                xs = AP(b2.tensor, b2.offset, [[1152, 128], [1, 256]])
                return xu, xs

            def emit_l2(bb, BACC, it, H1):
                gi, j, glen = git[it]
                if j == 0:
                    p2_state[0] = pp2.tile([128, 512], f32, tag="p2",
                                           name="p2t")
                cur = p2_state[0]
                lo = 256 * j
                if USE_FP8_L2:
                    for g in range(2):
                        w2d = W2D[:, 256 * g:256 * g + 256].rearrange(
                            "p (i m) -> p i m", i=2)
                        nc.tensor.matmul(cur[:, lo:lo + 256], w2d,
                                         H1[:, 512 * g:512 * g + 512].rearrange(
                                             "p (i c) -> p i c", i=2),
                                         start=(g == 0), stop=(g == 1),
                                         perf_mode=DR, skip_group_check=True)
                else:
                    for q in range(4):
                        nc.tensor.matmul(cur[:, lo:lo + 256],
                                         W2S[:, 128 * q:128 * q + 128],
                                         H1[:, 256 * q:256 * q + 256],
                                         start=(q == 0), stop=(q == 3),
                                         skip_group_check=True)
                if j != glen - 1:
                    return
                width = 256 * glen
                HJ = jpool.tile([128, 512], bf16, tag="hj")
                # spread most pair-activations onto the DVE (fused
                # lrelu+accumulate) to unload the ACT engine; the rest use
                # ACT with its built-in accumulator.
                # in fp8 mode P2 holds 64*(pre2 - b2); compute 64*lrelu2
                # (homogeneity) and divide acc by 64 on the host.
                on_dve = gi not in ACT_PAIRS and gi != 7
                if on_dve:
                    T2 = jpool.tile([128, 512], bf16, tag="t2")
                    if USE_FP8_L2:
                        nc.vector.tensor_scalar(
                            out=T2[:, 0:width], in0=cur[:, 0:width],
                            scalar1=0.01, scalar2=B2S64001,
                            op0=ALU.mult, op1=ALU.add)
                        nc.vector.scalar_tensor_tensor(
                            out=HJ[:, 0:width], in0=cur[:, 0:width],
                            scalar=B2S64, in1=T2[:, 0:width],
                            op0=ALU.add, op1=ALU.max,
                            accum_out=BACC[:, gi:gi + 1])
                    else:
                        nc.vector.tensor_scalar(
                            out=T2[:, 0:width], in0=cur[:, 0:width],
                            scalar1=B2S, scalar2=0.01,
                            op0=ALU.add, op1=ALU.mult)
                        nc.vector.scalar_tensor_tensor(
                            out=HJ[:, 0:width], in0=cur[:, 0:width],
                            scalar=B2S, in1=T2[:, 0:width],
                            op0=ALU.add, op1=ALU.max,
                            accum_out=BACC[:, gi:gi + 1])
                else:
                    nc.scalar.activation(HJ[:, 0:width], cur[:, 0:width],
                                         AF.Lrelu,
                                         bias=B2S64 if USE_FP8_L2 else B2S,
                                         scale=1.0, alpha=0.01,
                                         accum_out=BACC[:, gi:gi + 1])
                if gi == len(GROUPS) - 1:
                    nc.sync.dma_start(acc_o[bb], BACC[:])

            for it in range(N_ITERS):
                c0 = 4 * it
                A2 = apool.tile([128, 1024], bf16, tag="a2")
                pool_sub = POOL_SUB == 2 and it % 2 == 1
                eng_sub = nc.gpsimd if pool_sub else nc.vector
                eng_sub.tensor_tensor(
                    out=A2[:].rearrange("p (t s c) -> p t s c", t=2, s=2),
                    in0=sub_in0(), in1=sub_in1(c0), op=ALU.subtract)
                nc.vector.tensor_scalar(
                    out=A2[:].bitcast(u16), in0=A2[:].bitcast(u16),
                    scalar1=0x7FFF, scalar2=None, op0=ALU.bitwise_and)

                # ---- layer 1 ----
                P1 = pp1.tile([128, 1024], f32, tag="p1")
                nc.tensor.matmul(P1[:, 0:512], WBS, A2[:, 0:512],
                                 start=True, stop=False, skip_group_check=True)
                nc.tensor.matmul(P1[:, 512:1024], WBS, A2[:, 512:1024],
                                 start=True, stop=False, skip_group_check=True)
                for h in range(2):
                    for s in range(2):
                        pslice = P1[:, 512 * h + 256 * s:512 * h + 256 * s + 256]
                        if USE_FP8:
                            nc.tensor.matmul(pslice, WAS2, dr_rhs(h, c0, s),
                                             start=False, stop=True,
                                             perf_mode=DR,
                                             skip_group_check=True)
                        else:
                            xu, xs = bf_rhs(h, c0, s)
                            nc.tensor.matmul(pslice, WASB, xu,
                                             start=False, stop=False,
                                             skip_group_check=True)
                            nc.tensor.matmul(pslice, WASB, xs,
                                             start=False, stop=True,
                                             skip_group_check=True)

                # software pipeline: layer 2 trails layer 1 by SKEW
                # iterations in the PE queue, carried across batch
                # boundaries so the pipeline never drains mid-kernel.
                if len(pending) >= SKEW:
                    emit_l2(*pending.pop(0))

                H1 = hpool.tile([128, 1024], fp8 if USE_FP8_L2 else bf16,
                                tag="h1")
                if USE_FP8_L2:
                    nc.scalar.activation(H1[:], P1[:], AF.Lrelu, bias=B1S64,
                                         scale=64.0, alpha=0.01)
                else:
                    nc.scalar.activation(H1[:], P1[:], AF.Lrelu, bias=B1S,
                                         scale=1.0, alpha=0.01)
                pending.append((b, ACC, it, H1))

        for p in pending:
            emit_l2(*p)

    nc.compile()
    return nc


def _get_program():
    key = (N_ITERS, N_BATCH, USE_FP8, USE_FP8_L2, N_DVE_ACT2, SKEW,
           ACT_PAIRS, POOL_SUB, ACT_SOLOS,
           _os.environ.get("EPINN_JBUFS", "2"),
           _os.environ.get("EPINN_XBUFS", "2"))
    if key not in _CACHE:
        _CACHE[key] = _build_program()
    return _CACHE[key]


def _get_runner():
    """Build (once) a cached jitted SPMD executable for the program."""
    key = ("runner", N_ITERS, N_BATCH, N_RUN_CORES, USE_FP8, USE_FP8_L2,
           N_DVE_ACT2, ACT_PAIRS, POOL_SUB, ACT_SOLOS)
    if key in _CACHE:
        return _CACHE[key]
    import jax
    import jax.numpy as jnp
    import numpy as _np
    import concourse.mybir as mybir
    from jax.sharding import Mesh, PartitionSpec
    from jax.experimental.shard_map import shard_map
    from concourse import bass2jax
    from concourse.bass2jax import _bass_exec_p, partition_id_tensor

    bass2jax.install_neuronx_cc_hook()
    nc = _get_program()
    n_cores = N_RUN_CORES

    partition_name = (nc.partition_id_tensor.name
                      if nc.partition_id_tensor else None)
    in_names, out_names, out_avals, zero_shapes = [], [], [], []
    for alloc in nc.m.functions[0].allocations:
        if not isinstance(alloc, mybir.MemoryLocationSet):
            continue
        name = alloc.memorylocations[0].name
        if alloc.kind == "ExternalInput":
            if name != partition_name:
                in_names.append(name)
        elif alloc.kind == "ExternalOutput":
            out_names.append(name)
            shape = tuple(alloc.tensor_shape)
            dtype = mybir.dt.np(alloc.dtype)
            out_avals.append(jax.core.ShapedArray(shape, dtype))
            zero_shapes.append((shape, dtype))
    n_params = len(in_names)
    n_outs = len(out_avals)
    all_in_names = list(in_names) + list(out_names)
    if partition_name is not None:
        all_in_names.append(partition_name)
    donate = tuple(range(n_params, n_params + n_outs))

    def _body(*args):
        operands = list(args)
        if partition_name is not None:
            operands.append(partition_id_tensor())
        outs = _bass_exec_p.bind(
            *operands, out_avals=tuple(out_avals), in_names=tuple(all_in_names),
            out_names=tuple(out_names), lowering_input_output_aliases=(),
            sim_require_finite=True, sim_require_nnan=True, nc=nc)
        return tuple(outs)

    devices = jax.devices()[:n_cores]
    mesh = Mesh(_np.asarray(devices), ("core",))
    in_specs = (PartitionSpec("core"),) * (n_params + n_outs)
    out_specs = (PartitionSpec("core"),) * len(out_names)
    sharded = jax.jit(
        shard_map(_body, mesh=mesh, in_specs=in_specs, out_specs=out_specs,
                  check_rep=False),
        donate_argnums=donate, keep_unused=True)

    def run(in_maps):
        concat_in = [
            np.concatenate([np.asarray(in_maps[c][nm]) for c in range(n_cores)],
                           axis=0)
            for nm in in_names
        ]
        concat_zeros = [np.zeros((n_cores * s[0], *s[1:]), d)
                        for (s, d) in zero_shapes]
        out_arrs = sharded(*concat_in, *concat_zeros)
        return [
            {nm: np.asarray(out_arrs[i]).reshape(n_cores, *out_avals[i].shape)[c]
             for i, nm in enumerate(out_names)}
            for c in range(n_cores)
        ]

    _CACHE[key] = run
    return run


def _prep_in_maps(emb, tw, w1, b1, w2, b2):
    import ml_dtypes
    bfl = ml_dtypes.bfloat16
    f8 = ml_dtypes.float8_e4m3

    emb = np.asarray(emb, np.float32)
    tw = np.asarray(tw, np.float32)
    w1 = np.asarray(w1, np.float32)
    b1v = np.asarray(b1, np.float32)
    w2f = np.asarray(w2, np.float32)
    b2v = np.asarray(b2, np.float32)

    w1bt = np.ascontiguousarray(w1[:, 64:].T)          # [64f, 64d]
    w1at = np.ascontiguousarray(0.5 * w1[:, :64].T)    # [64f, 64d]
    wbs = np.zeros((128, 128), np.float32)
    wbs[0:64, 0:64] = w1bt
    wbs[64:128, 64:128] = w1bt
    was = np.zeros((128, 128), np.float32)
    was[0:64, 0:64] = w1at
    was[64:128, 64:128] = w1at
    w2s = np.zeros((128, 512), np.float32)
    for q in range(4):
        w2s[0:64, 128 * q + 32 * q:128 * q + 32 * q + 16] = w2f.T
        w2s[64:128, 128 * q + 32 * q + 16:128 * q + 32 * q + 32] = w2f.T
    wp = np.concatenate([wbs, w2s, was], axis=1)       # [128, 768]
    was8 = np.concatenate([was, was], axis=1)          # [128, 256]
    b1d = np.concatenate([b1v, b1v])
    b2d = np.tile(b2v, 8)
    bp = np.stack([b1d, b2d, 64.0 * b1d, 64.0 * b2d, 0.64 * b2d,
                   np.zeros(128, np.float32), np.zeros(128, np.float32),
                   np.zeros(128, np.float32)], axis=1)  # [128, 8]
    w2d = np.zeros((128, 2, 2, 128), np.float32)       # [p, g, i, m]
    for q in range(4):
        w2d[:, q // 2, q % 2, :] = w2s[:, 128 * q:128 * q + 128]
    w2d = w2d.reshape(128, 512)
    wk = np.zeros((128, 1168 * 2), np.uint8)
    wk[:, 0:1536] = wp.astype(bfl).view(np.uint8)
    wk[:, 1536:1792] = was8.astype(f8).view(np.uint8)
    wk[:, 1792:1824] = bp.astype(np.float32).view(np.uint8)
    wk[:, 1824:2336] = w2d.astype(f8).view(np.uint8)
    wk = wk.view(bfl)                                  # [128, 1168]

    twp = tw[:-1].reshape(L, D)
    idx1 = (np.arange(320) + 1) % 256
    idx65 = (np.arange(320) + 65) % 256
    idx2 = (np.arange(320) + 2) % 256
    idx66 = (np.arange(320) + 66) % 256

    shared = {"wk": wk}
    in_maps = []
    for c in range(N_CORES):
        xall = np.zeros((N_BATCH, 128, 1152), np.float32)
        for b in range(N_BATCH):
            gb = c * B_PER_CORE + b
            x2t = (emb[gb, :L * D].reshape(L, D) * twp).T  # [64, 256]
            x2t2 = np.concatenate([x2t, x2t], axis=0)      # [128, 256]
            xall[b, :, 0:256] = x2t2
            xall[b, :, 256:512] = x2t2
            xall[b, 0:64, 512:832] = x2t[:, idx1]
            xall[b, 64:128, 512:832] = x2t[:, idx65]
            xall[b, 0:64, 832:1152] = x2t[:, idx2]
            xall[b, 64:128, 832:1152] = x2t[:, idx66]
        xf8 = np.zeros((N_BATCH, 128, 1152), np.float32)
        xf8[:, :, 0:256] = xall[:, :, 0:256]
        xf8[:, :, 256:576] = xall[:, :, 512:832]    # XSo
        xf8[:, :, 576:832] = xall[:, :, 0:256]      # X2T again
        xf8[:, :, 832:1152] = xall[:, :, 832:1152]  # XSe
        m = dict(shared)
        m["xall"] = xall.astype(bfl)
        m["xf8"] = xf8.astype(f8)
        in_maps.append(m)
    return in_maps


_FP8_SCALED = USE_FP8_L2


def _s128_exact(emb, tw, w1, b1, w2, b2, w3):
    """Exact per-batch sum of s over the 256 ordered (i, i+128) pairs
    (w3 . lrelu2, no b3)."""
    emb = np.asarray(emb, np.float64)
    tw = np.asarray(tw, np.float64)
    x2 = emb[:, :L * D].reshape(-1, L, D) * tw[:-1].reshape(L, D)  # [B,L,D]
    xj = np.roll(x2, -128, axis=1)
    feat = np.concatenate([0.5 * (x2 + xj), np.abs(x2 - xj)], axis=-1)
    pre1 = feat @ np.asarray(w1, np.float64).T + np.asarray(b1, np.float64)
    h1 = np.where(pre1 > 0, pre1, 0.01 * pre1)
    pre2 = h1 @ np.asarray(w2, np.float64).T + np.asarray(b2, np.float64)
    h2 = np.where(pre2 > 0, pre2, 0.01 * pre2)
    return (h2 @ np.asarray(w3, np.float64)[0]).sum(axis=1)  # [B]


def _finish(core_results, emb, tw, tb, w1, b1, w2, b2, w3, b3, scale):
    emb = np.asarray(emb, np.float64)
    tw = np.asarray(tw, np.float64)
    x1 = emb @ tw + float(tb[0])  # [32]
    w3v = np.asarray(w3, np.float32)[0]       # [16]
    w3t = np.tile(w3v, 8)                     # [128]
    s128 = _s128_exact(emb, tw, w1, b1, w2, b2, w3)  # [32]
    out = np.zeros(32, np.float32)
    for c in range(len(core_results)):
        acc = core_results[c]["acc_o"]        # [4, 128, n_acc]
        for b in range(N_BATCH):
            a = acc[b] / 64.0 if _FP8_SCALED else acc[b]
            tot = float(w3t @ a.sum(axis=1))
            gb = c * B_PER_CORE + b
            # o=128 pairs were enumerated twice; remove one copy exactly
            tot -= 0.5 * float(s128[gb])
            out[gb] = (x1[gb]
                       + float(scale[0]) * (tot + float(b3[0]) * NPAIRS))
    return out


def kernel(emb, tw, tb, w1, b1, w2, b2, w3, b3, scale):
    run = _get_runner()
    in_maps = _prep_in_maps(emb, tw, w1, b1, w2, b2)
    core_results = run(in_maps[:N_RUN_CORES])
    return _finish(core_results, emb, tw, tb, w1, b1, w2, b2, w3, b3, scale)


# revision 42
# speedup vs baseline: 1.0240x; 1.0039x over previous
"""Trainium2 Bass kernel for nn_EpiNN_aaindex (pairwise-MLP GNN reduction).

Math (per batch b):
  x1 = emb@tw + tb                               (host)
  X[i,d] = emb[i*64+d] * tw[i*64+d]              (L=256, D=64; host)
  s_ij = MLP(concat[(x_i+x_j)/2, |x_i-x_j|])     (64->16->1, LeakyReLU 0.01)
  out_b = x1 + scale * sum_{i<j} s_ij

Strategy: 8 cores, 4 batches/core (data parallel over B=32).
Exact upper-triangle enumeration via cyclic offsets o=1..128:
pairs (i, (i+o) mod 256) for o=1..127 cover each unordered pair once;
o=128 covers each of its 128 pairs twice -- the host recomputes that one
offset's 256-pair term exactly in numpy and subtracts half of it.

Host prep per batch: X2T = (emb[:, :-1].reshape(L, D) * tw2d).T  [64, 256]
  XALL [128, 1152] bf16: [X2T|X2T (512) | XSo (320) | XSe (320)]
    XSo: top half rows = X2T cyclic-shift 1, bottom = shift 65
    XSe: shifts 2 / 66
  XF8 [128, 1152] fp8e4: [X2T(256)|XSo(320) | X2T(256)|XSe(320)] - for the
    DoubleRow S-term matmuls.

Device per iteration it (c0 = 4*it), quarters q0..q3 of 256 pairs each
(q0 = offsets 4it+1|+65, q1 = 4it+3|+67, q2 = 4it+2|+66, q3 = 4it+4|+68):
  A2 [128, 1024] bf16 = |XU - XS[c0 window]|: one fused tensor_tensor
    subtract (on the DVE for even iterations, on GPSIMD for odd ones --
    walrus accepts TensorTensor on Pool, not TensorScalarPtr) + one DVE
    bitand (u16 4x mode) for the abs.
  P1 [128, 1024] f32 psum:
    bf16 matmul  P1 += WBSstack @ A2            (128-contract, 2x512 cols)
    fp8 DoubleRow P1 += WAS@X2T + WAS@XS-window (4 mms, 256 cols @ 2x rate)
  H1 = Lrelu(P1 + b1)  [128, 1024] bf16   (ACT, bias=B1S)
  P2: layer-2 outputs of iteration PAIRS share one [128, 1024]-capable psum
    tile; per iteration 4 bf16 matmuls place each quarter's 16-dim output at
    psum rows 32q (top) / 32q+16 (bottom) -> all 128 rows used.
  act2 = Lrelu(P2 + b2) summed over iteration pairs: 2 of the 8 pair
    groups on the ACT engine (activation + accum_out), 6 on the DVE
    (tensor_scalar + scalar_tensor_tensor max(x, 0.01x) with accum_out)
    so that ACT/DVE/PE/Pool all sit near 79-86% busy.

Layer-2 weights stay bf16: quantizing w2 to fp8 injects noise into pre2,
and E[lrelu(x + noise)] > lrelu(x) (convexity) -> a systematic positive
output bias (~+1.2 measured). fp8 on the layer-1 S-term is bias-free.

Software-pipelined emission with 3-iteration skew: layer-2 of iteration
k-3 is emitted after layer-1 of iteration k so the PE queue never stalls
on act1.

Final combine on host:
  out = x1 + scale*(w3tile . ACC - 0.5*s128_exact + b3*32640).
"""
import numpy as np

L, D = 256, 64
B_PER_CORE = 4
N_CORES = 8
NPAIRS = 32640  # 256*255/2

_CACHE = {}
import os as _os
_os  # env knobs read in _build_program too
N_ITERS = int(_os.environ.get("EPINN_ITERS", "16"))
N_BATCH = int(_os.environ.get("EPINN_BATCH", str(B_PER_CORE)))
N_RUN_CORES = int(_os.environ.get("EPINN_CORES", str(N_CORES)))
USE_FP8 = _os.environ.get("EPINN_FP8", "1") == "1"
USE_FP8_L2 = _os.environ.get("EPINN_FP8_L2", "0") == "1"
N_DVE_ACT2 = int(_os.environ.get("EPINN_DVE_ACT2", "3"))
_ACT_PAIR_SETS = {
    0: (),
    1: (3,),
    2: (1, 5),
    3: (0, 3, 6),
    4: (0, 2, 4, 6),
    5: (0, 1, 3, 5, 6),
    7: (0, 1, 2, 3, 4, 5, 6),
    10: (0,), 11: (1,), 12: (2,), 14: (4,), 15: (5,), 16: (6,),
}
ACT_PAIRS = _ACT_PAIR_SETS[int(_os.environ.get("EPINN_ACT_PAIRS", "1"))]
# 0 = A2 subtract always on DVE, 2 = subtract on Pool for odd iterations
# (walrus accepts TensorTensor on Pool but rejects TensorScalarPtr there)
POOL_SUB = int(_os.environ.get("EPINN_POOL_SUB", "2"))
ACT_SOLOS = int(_os.environ.get("EPINN_ACT_SOLOS", "2"))
SKEW = int(_os.environ.get("EPINN_SKEW", "3"))


def _build_program():
    import concourse.bacc as bacc
    import concourse.mybir as mybir
    import concourse.tile as tile
    from concourse.ap import AP
    from contextlib import ExitStack

    f32 = mybir.dt.float32
    bf16 = mybir.dt.bfloat16
    fp8 = mybir.dt.float8e4
    u16 = mybir.dt.uint16
    AF = mybir.ActivationFunctionType
    ALU = mybir.AluOpType
    DR = mybir.MatmulPerfMode.DoubleRow

    nc = bacc.Bacc("TRN2", target_bir_lowering=False, debug=False,
                   num_devices=N_CORES)

    # ---- DRAM parameters (per core) ----
    xall_d = nc.declare_dram_parameter("xall", [N_BATCH, 128, 1152], bf16,
                                       isOutput=False)
    xf8_d = nc.declare_dram_parameter("xf8", [N_BATCH, 128, 1152], fp8,
                                      isOutput=False)
    wk_d = nc.declare_dram_parameter("wk", [128, 1168], bf16, isOutput=False)

    # acc columns: one per iteration pair. The double-counted o=128 stream
    # (iteration 15) no longer needs a separable column: the host recomputes
    # that term exactly and subtracts it.
    assert N_ITERS == 16
    GROUPS = [(0, 1), (2, 3), (4, 5), (6, 7), (8, 9), (10, 11), (12, 13),
              (14, 15)]
    n_acc = len(GROUPS)
    acc_o = nc.declare_dram_parameter("acc_o", [N_BATCH, 128, n_acc], f32,
                                      isOutput=True)

    with tile.TileContext(nc) as tc, ExitStack() as ctx:
        cpool = ctx.enter_context(tc.tile_pool(name="consts", bufs=1))
        XBUFS = int(_os.environ.get("EPINN_XBUFS", "2"))
        xpool = ctx.enter_context(tc.tile_pool(name="xbufs", bufs=XBUFS))
        apool = ctx.enter_context(tc.tile_pool(name="abufs", bufs=2 + SKEW))
        hpool = ctx.enter_context(tc.tile_pool(name="hbufs", bufs=2 + SKEW))
        JBUFS = int(_os.environ.get("EPINN_JBUFS", "2"))
        jpool = ctx.enter_context(tc.tile_pool(name="junk", bufs=JBUFS))
        opool = ctx.enter_context(tc.tile_pool(name="outs", bufs=2))
        pp1 = ctx.enter_context(tc.tile_pool(name="p1", bufs=3, space="PSUM"))
        pp2 = ctx.enter_context(tc.tile_pool(name="p2", bufs=2, space="PSUM"))

        DUM = cpool.tile([1, 2], f32)
        nc.gpsimd.memset(DUM[:], 0.0)
        nc.scalar.activation(DUM[:], DUM[:], AF.Lrelu, scale=1.0, alpha=0.01)

        # batch-0 inputs first: the first DVE subtract only needs XALL, so
        # its DMA leads; weights follow (needed ~1.5us later by the first mm).
        XT0 = xpool.tile([128, 1152], bf16, tag="xall", name="xall0")
        nc.sync.dma_start(XT0[:], xall_d[0])
        WK = cpool.tile([128, 1168], bf16)
        nc.sync.dma_start(WK[:], wk_d[:])
        XF0 = xpool.tile([128, 1152], fp8, tag="xf8", name="xf80")
        if USE_FP8:
            nc.sync.dma_start(XF0[:], xf8_d[0])
        WBS = WK[:, 0:128]
        W2S = WK[:, 128:640]
        WASB = WK[:, 640:768]
        WAS8 = WK[:, 768:896].bitcast(fp8)
        BP = WK[:, 896:912].bitcast(f32)
        B1S = BP[:, 0:1]
        B2S = BP[:, 1:2]
        B1S64 = BP[:, 2:3]
        B2S64 = BP[:, 3:4]
        B2S64001 = BP[:, 4:5]
        WAS2 = WAS8.rearrange("p (i m) -> p i m", i=2)
        W2D = WK[:, 912:1168].bitcast(fp8)  # [128, 512] = 2 groups x [2, 128]

        git = {it: (gi, j, len(g))
               for gi, g in enumerate(GROUPS) for j, it in enumerate(g)}
        p2_state = [None]  # current group psum tile
        pending = []       # [(b, ACC, it, H1)] layer-2 backlog (SKEW deep)

        for b in range(N_BATCH):
            if b == 0:
                XALL, XF8 = XT0, XF0
            else:
                XALL = xpool.tile([128, 1152], bf16, tag="xall")
                XF8 = xpool.tile([128, 1152], fp8, tag="xf8")
                nc.sync.dma_start(XALL[:], xall_d[b])
                if USE_FP8:
                    nc.sync.dma_start(XF8[:], xf8_d[b])

            ACC = opool.tile([128, n_acc], f32, tag="acc")

            def sub_in0():
                base = XALL[:, 0:512]
                return AP(base.tensor, base.offset,
                          [[1152, 128], [0, 2], [256, 2], [1, 256]])

            def sub_in1(c0):
                base = XALL[:, 512 + c0:512 + c0 + 1]
                return AP(base.tensor, base.offset,
                          [[1152, 128], [320, 2], [2, 2], [1, 256]])

            def dr_rhs(h, c0, s):
                base = XF8[:, 576 * h:576 * h + 1]
                return AP(base.tensor, base.offset,
                          [[1152, 128], [256 + c0 + 2 * s, 2], [1, 256]])

            def bf_rhs(h, c0, s):
                # fallback (no fp8): [X2T-copy | XS-window] via two mms
                base = XALL[:, 0:1]
                xu = AP(base.tensor, base.offset, [[1152, 128], [1, 256]])
                b2 = XALL[:, 512 + 320 * h + c0 + 2 * s:]
                xs = AP(b2.tensor, b2.offset, [[1152, 128], [1, 256]])
                return xu, xs

            def emit_l2(bb, BACC, it, H1):
                gi, j, glen = git[it]
                if j == 0:
                    p2_state[0] = pp2.tile([128, 512], f32, tag="p2",
                                           name="p2t")
                cur = p2_state[0]
                lo = 256 * j
                if USE_FP8_L2:
                    for g in range(2):
                        w2d = W2D[:, 256 * g:256 * g + 256].rearrange(
                            "p (i m) -> p i m", i=2)
                        nc.tensor.matmul(cur[:, lo:lo + 256], w2d,
                                         H1[:, 512 * g:512 * g + 512].rearrange(
                                             "p (i c) -> p i c", i=2),
                                         start=(g == 0), stop=(g == 1),
                                         perf_mode=DR, skip_group_check=True)
                else:
                    for q in range(4):
                        nc.tensor.matmul(cur[:, lo:lo + 256],
                                         W2S[:, 128 * q:128 * q + 128],
                                         H1[:, 256 * q:256 * q + 256],
                                         start=(q == 0), stop=(q == 3),
                                         skip_group_check=True)
                if j != glen - 1:
                    return
                width = 256 * glen
                HJ = jpool.tile([128, 512], bf16, tag="hj")
                # spread most pair-activations onto the DVE (fused
                # lrelu+accumulate) to unload the ACT engine; the rest use
                # ACT with its built-in accumulator.
                # in fp8 mode P2 holds 64*(pre2 - b2); compute 64*lrelu2
                # (homogeneity) and divide acc by 64 on the host.
                on_dve = gi not in ACT_PAIRS and gi != 7
                if on_dve:
                    T2 = jpool.tile([128, 512], bf16, tag="t2")
                    if USE_FP8_L2:
                        nc.vector.tensor_scalar(
                            out=T2[:, 0:width], in0=cur[:, 0:width],
                            scalar1=0.01, scalar2=B2S64001,
                            op0=ALU.mult, op1=ALU.add)
                        nc.vector.scalar_tensor_tensor(
                            out=HJ[:, 0:width], in0=cur[:, 0:width],
                            scalar=B2S64, in1=T2[:, 0:width],
                            op0=ALU.add, op1=ALU.max,
                            accum_out=BACC[:, gi:gi + 1])
                    else:
                        nc.vector.tensor_scalar(
                            out=T2[:, 0:width], in0=cur[:, 0:width],
                            scalar1=B2S, scalar2=0.01,
                            op0=ALU.add, op1=ALU.mult)
                        nc.vector.scalar_tensor_tensor(
                            out=HJ[:, 0:width], in0=cur[:, 0:width],
                            scalar=B2S, in1=T2[:, 0:width],
                            op0=ALU.add, op1=ALU.max,
                            accum_out=BACC[:, gi:gi + 1])
                else:
                    nc.scalar.activation(HJ[:, 0:width], cur[:, 0:width],
                                         AF.Lrelu,
                                         bias=B2S64 if USE_FP8_L2 else B2S,
                                         scale=1.0, alpha=0.01,
                                         accum_out=BACC[:, gi:gi + 1])
                if gi == len(GROUPS) - 1:
                    nc.sync.dma_start(acc_o[bb], BACC[:])

            for it in range(N_ITERS):
                c0 = 4 * it
                A2 = apool.tile([128, 1024], bf16, tag="a2")
                pool_sub = POOL_SUB == 2 and it % 2 == 1
                eng_sub = nc.gpsimd if pool_sub else nc.vector
                eng_sub.tensor_tensor(
                    out=A2[:].rearrange("p (t s c) -> p t s c", t=2, s=2),
                    in0=sub_in0(), in1=sub_in1(c0), op=ALU.subtract)
                nc.vector.tensor_scalar(
                    out=A2[:].bitcast(u16), in0=A2[:].bitcast(u16),
                    scalar1=0x7FFF, scalar2=None, op0=ALU.bitwise_and)

                # ---- layer 1 ----
                P1 = pp1.tile([128, 1024], f32, tag="p1")
                nc.tensor.matmul(P1[:, 0:512], WBS, A2[:, 0:512],
                                 start=True, stop=False, skip_group_check=True)
                nc.tensor.matmul(P1[:, 512:1024], WBS, A2[:, 512:1024],
                                 start=True, stop=False, skip_group_check=True)
                for h in range(2):
                    for s in range(2):
                        pslice = P1[:, 512 * h + 256 * s:512 * h + 256 * s + 256]
                        if USE_FP8:
                            nc.tensor.matmul(pslice, WAS2, dr_rhs(h, c0, s),
                                             start=False, stop=True,
                                             perf_mode=DR,
                                             skip_group_check=True)
                        else:
                            xu, xs = bf_rhs(h, c0, s)
                            nc.tensor.matmul(pslice, WASB, xu,
                                             start=False, stop=False,
                                             skip_group_check=True)
                            nc.tensor.matmul(pslice, WASB, xs,
                                             start=False, stop=True,
                                             skip_group_check=True)

                # software pipeline: layer 2 trails layer 1 by SKEW
                # iterations in the PE queue, carried across batch
                # boundaries so the pipeline never drains mid-kernel.
                if len(pending) >= SKEW:
                    emit_l2(*pending.pop(0))

                H1 = hpool.tile([128, 1024], fp8 if USE_FP8_L2 else bf16,
                                tag="h1")
                if USE_FP8_L2:
                    nc.scalar.activation(H1[:], P1[:], AF.Lrelu, bias=B1S64,
                                         scale=64.0, alpha=0.01)
                else:
                    nc.scalar.activation(H1[:], P1[:], AF.Lrelu, bias=B1S,
                                         scale=1.0, alpha=0.01)
                pending.append((b, ACC, it, H1))

        for p in pending:
            emit_l2(*p)

    nc.compile()
    return nc


def _get_program():
    key = (N_ITERS, N_BATCH, USE_FP8, USE_FP8_L2, N_DVE_ACT2, SKEW,
           ACT_PAIRS, POOL_SUB, ACT_SOLOS,
           _os.environ.get("EPINN_JBUFS", "2"),
           _os.environ.get("EPINN_XBUFS", "2"))
    if key not in _CACHE:
        _CACHE[key] = _build_program()
    return _CACHE[key]


def _get_runner():
    """Build (once) a cached jitted SPMD executable for the program."""
    key = ("runner", N_ITERS, N_BATCH, N_RUN_CORES, USE_FP8, USE_FP8_L2,
           N_DVE_ACT2, ACT_PAIRS, POOL_SUB, ACT_SOLOS)
    if key in _CACHE:
        return _CACHE[key]
    import jax
    import jax.numpy as jnp
    import numpy as _np
    import concourse.mybir as mybir
    from jax.sharding import Mesh, PartitionSpec
    from jax.experimental.shard_map import shard_map
    from concourse import bass2jax
    from concourse.bass2jax import _bass_exec_p, partition_id_tensor

    bass2jax.install_neuronx_cc_hook()
    nc = _get_program()
    n_cores = N_RUN_CORES

    partition_name = (nc.partition_id_tensor.name
                      if nc.partition_id_tensor else None)
    in_names, out_names, out_avals, zero_shapes = [], [], [], []
    for alloc in nc.m.functions[0].allocations:
        if not isinstance(alloc, mybir.MemoryLocationSet):
            continue
        name = alloc.memorylocations[0].name
        if alloc.kind == "ExternalInput":
            if name != partition_name:
                in_names.append(name)
        elif alloc.kind == "ExternalOutput":
            out_names.append(name)
            shape = tuple(alloc.tensor_shape)
            dtype = mybir.dt.np(alloc.dtype)
            out_avals.append(jax.core.ShapedArray(shape, dtype))
            zero_shapes.append((shape, dtype))
    n_params = len(in_names)
    n_outs = len(out_avals)
    all_in_names = list(in_names) + list(out_names)
    if partition_name is not None:
        all_in_names.append(partition_name)
    donate = tuple(range(n_params, n_params + n_outs))

    def _body(*args):
        operands = list(args)
        if partition_name is not None:
            operands.append(partition_id_tensor())
        outs = _bass_exec_p.bind(
            *operands, out_avals=tuple(out_avals), in_names=tuple(all_in_names),
            out_names=tuple(out_names), lowering_input_output_aliases=(),
            sim_require_finite=True, sim_require_nnan=True, nc=nc)
        return tuple(outs)

    devices = jax.devices()[:n_cores]
    mesh = Mesh(_np.asarray(devices), ("core",))
    in_specs = (PartitionSpec("core"),) * (n_params + n_outs)
    out_specs = (PartitionSpec("core"),) * len(out_names)
    sharded = jax.jit(
        shard_map(_body, mesh=mesh, in_specs=in_specs, out_specs=out_specs,
                  check_rep=False),
        donate_argnums=donate, keep_unused=True)

    def run(in_maps):
        concat_in = [
            np.concatenate([np.asarray(in_maps[c][nm]) for c in range(n_cores)],
                           axis=0)
            for nm in in_names
        ]
        concat_zeros = [np.zeros((n_cores * s[0], *s[1:]), d)
                        for (s, d) in zero_shapes]
        out_arrs = sharded(*concat_in, *concat_zeros)
        return [
            {nm: np.asarray(out_arrs[i]).reshape(n_cores, *out_avals[i].shape)[c]
             for i, nm in enumerate(out_names)}
            for c in range(n_cores)
        ]

    _CACHE[key] = run
    return run


def _prep_in_maps(emb, tw, w1, b1, w2, b2):
    import ml_dtypes
    bfl = ml_dtypes.bfloat16
    f8 = ml_dtypes.float8_e4m3

    emb = np.asarray(emb, np.float32)
    tw = np.asarray(tw, np.float32)
    w1 = np.asarray(w1, np.float32)
    b1v = np.asarray(b1, np.float32)
    w2f = np.asarray(w2, np.float32)
    b2v = np.asarray(b2, np.float32)

    w1bt = np.ascontiguousarray(w1[:, 64:].T)          # [64f, 64d]
    w1at = np.ascontiguousarray(0.5 * w1[:, :64].T)    # [64f, 64d]
    wbs = np.zeros((128, 128), np.float32)
    wbs[0:64, 0:64] = w1bt
    wbs[64:128, 64:128] = w1bt
    was = np.zeros((128, 128), np.float32)
    was[0:64, 0:64] = w1at
    was[64:128, 64:128] = w1at
    w2s = np.zeros((128, 512), np.float32)
    for q in range(4):
        w2s[0:64, 128 * q + 32 * q:128 * q + 32 * q + 16] = w2f.T
        w2s[64:128, 128 * q + 32 * q + 16:128 * q + 32 * q + 32] = w2f.T
    wp = np.concatenate([wbs, w2s, was], axis=1)       # [128, 768]
    was8 = np.concatenate([was, was], axis=1)          # [128, 256]
    b1d = np.concatenate([b1v, b1v])
    b2d = np.tile(b2v, 8)
    bp = np.stack([b1d, b2d, 64.0 * b1d, 64.0 * b2d, 0.64 * b2d,
                   np.zeros(128, np.float32), np.zeros(128, np.float32),
                   np.zeros(128, np.float32)], axis=1)  # [128, 8]
    w2d = np.zeros((128, 2, 2, 128), np.float32)       # [p, g, i, m]
    for q in range(4):
        w2d[:, q // 2, q % 2, :] = w2s[:, 128 * q:128 * q + 128]
    w2d = w2d.reshape(128, 512)
    wk = np.zeros((128, 1168 * 2), np.uint8)
    wk[:, 0:1536] = wp.astype(bfl).view(np.uint8)
    wk[:, 1536:1792] = was8.astype(f8).view(np.uint8)
    wk[:, 1792:1824] = bp.astype(np.float32).view(np.uint8)
    wk[:, 1824:2336] = w2d.astype(f8).view(np.uint8)
    wk = wk.view(bfl)                                  # [128, 1168]

    twp = tw[:-1].reshape(L, D)
    idx1 = (np.arange(320) + 1) % 256
    idx65 = (np.arange(320) + 65) % 256
    idx2 = (np.arange(320) + 2) % 256
    idx66 = (np.arange(320) + 66) % 256

    shared = {"wk": wk}
    in_maps = []
    for c in range(N_CORES):
        xall = np.zeros((N_BATCH, 128, 1152), np.float32)
        for b in range(N_BATCH):
            gb = c * B_PER_CORE + b
            x2t = (emb[gb, :L * D].reshape(L, D) * twp).T  # [64, 256]
            x2t2 = np.concatenate([x2t, x2t], axis=0)      # [128, 256]
            xall[b, :, 0:256] = x2t2
            xall[b, :, 256:512] = x2t2
            xall[b, 0:64, 512:832] = x2t[:, idx1]
            xall[b, 64:128, 512:832] = x2t[:, idx65]
            xall[b, 0:64, 832:1152] = x2t[:, idx2]
            xall[b, 64:128, 832:1152] = x2t[:, idx66]
        xf8 = np.zeros((N_BATCH, 128, 1152), np.float32)
        xf8[:, :, 0:256] = xall[:, :, 0:256]
        xf8[:, :, 256:576] = xall[:, :, 512:832]    # XSo
        xf8[:, :, 576:832] = xall[:, :, 0:256]      # X2T again
        xf8[:, :, 832:1152] = xall[:, :, 832:1152]  # XSe
        m = dict(shared)
        m["xall"] = xall.astype(bfl)
        m["xf8"] = xf8.astype(f8)
        in_maps.append(m)
    return in_maps


_FP8_SCALED = USE_FP8_L2


def _s128_exact(emb, tw, w1, b1, w2, b2, w3):
    """Exact per-batch sum of s over the 256 ordered (i, i+128) pairs
    (w3 . lrelu2, no b3)."""
    emb = np.asarray(emb, np.float64)
    tw = np.asarray(tw, np.float64)
    x2 = emb[:, :L * D].reshape(-1, L, D) * tw[:-1].reshape(L, D)  # [B,L,D]
    xj = np.roll(x2, -128, axis=1)
    feat = np.concatenate([0.5 * (x2 + xj), np.abs(x2 - xj)], axis=-1)
    pre1 = feat @ np.asarray(w1, np.float64).T + np.asarray(b1, np.float64)
    h1 = np.where(pre1 > 0, pre1, 0.01 * pre1)
    pre2 = h1 @ np.asarray(w2, np.float64).T + np.asarray(b2, np.float64)
    h2 = np.where(pre2 > 0, pre2, 0.01 * pre2)
    return (h2 @ np.asarray(w3, np.float64)[0]).sum(axis=1)  # [B]


def _finish(core_results, emb, tw, tb, w1, b1, w2, b2, w3, b3, scale):
    emb = np.asarray(emb, np.float64)
    tw = np.asarray(tw, np.float64)
    x1 = emb @ tw + float(tb[0])  # [32]
    w3v = np.asarray(w3, np.float32)[0]       # [16]
    w3t = np.tile(w3v, 8)                     # [128]
    s128 = _s128_exact(emb, tw, w1, b1, w2, b2, w3)  # [32]
    out = np.zeros(32, np.float32)
    for c in range(len(core_results)):
        acc = core_results[c]["acc_o"]        # [4, 128, n_acc]
        for b in range(N_BATCH):
            a = acc[b] / 64.0 if _FP8_SCALED else acc[b]
            tot = float(w3t @ a.sum(axis=1))
            gb = c * B_PER_CORE + b
            # o=128 pairs were enumerated twice; remove one copy exactly
            tot -= 0.5 * float(s128[gb])
            out[gb] = (x1[gb]
                       + float(scale[0]) * (tot + float(b3[0]) * NPAIRS))
    return out


def kernel(emb, tw, tb, w1, b1, w2, b2, w3, b3, scale):
    run = _get_runner()
    in_maps = _prep_in_maps(emb, tw, w1, b1, w2, b2)
    core_results = run(in_maps[:N_RUN_CORES])
    return _finish(core_results, emb, tw, tb, w1, b1, w2, b2, w3, b3, scale)
